# revision 27
# baseline (speedup 1.0000x reference)
"""Trainium2 Bass kernel for nn_Drug_PNAConv (GNN message passing, PNAConv).

v3c strategy:
  - Nodes partitioned by destination across 8 cores; host bins edges by dest
    degree into fixed chunks (g nodes x degree d), slots k-major ([d, g]).
  - Source features pre-gathered ON HOST into a contiguous feature-major bf16
    stream xjT [128, S]; no device gather at all.
  - Edge phase per chunk: p1 = Wbea.bond + Wxj.xj + Wxi.xi (PSUM), h1 = relu
    (scalar evac), p2 = W2BD.h1.  Segmented SUM and SUMSQ via single matmuls
    with stride-0 output APs (PSUM self-accumulation); min/max via DVE
    tensor_reduce directly from p2 PSUM (raw, pre-bias).
  - b_pre2 never applied per-edge: sum/mean/min/max offsets are folded into
    per-degree-class correction vectors (rank-1 ones matmuls); variance is
    shift-invariant so the std chain uses raw moments.
  - Degree scalers (identity/amplification/attenuation) folded into
    per-degree-class combined post weights W(d,a) = W1(0,a) + amp_d*W1(1,a)
    + att_d*W1(2,a); mean folded into the s-weight (Wsm = Wc0 + Wc1/d).
    d=0 and d=1 collapse further (std=sqrt(eps) exactly).
  - LayerNorm mean-centering folded into W_lin on host (P_c = I - 11^T/128);
    variance via ones-column matmul; rstd broadcast via ones-row matmul.
  - Edge and post phases interleaved per 512-node block for engine overlap.
"""

import os
import sys

for _p in ("/opt/trn_rl_repo", os.path.expanduser("~/.axon_site/_ro/trn_rl_repo")):
    if os.path.isdir(_p) and _p not in sys.path:
        sys.path.insert(0, _p)

import numpy as np

import concourse.bass as bass
import concourse.bacc as bacc
import concourse.mybir as mybir
import concourse.tile as tile
from concourse.bass_utils import run_bass_kernel_spmd
from concourse.masks import make_identity

F32 = mybir.dt.float32
BF16 = mybir.dt.bfloat16
AF = mybir.ActivationFunctionType
OP = mybir.AluOpType
AX = mybir.AxisListType

N_CORES = 8
H = 128
T = 4
F_IN = 32
EC = 16
EPS = 1e-5
GROUP_COLS = 2048

_DEG_HIST = np.array([0.0, 5000.0, 20000.0, 25000.0, 10000.0])
_BINS = np.arange(_DEG_HIST.size)
AVG_DEG_LOG = float((np.log(_BINS + 1.0) * _DEG_HIST).sum() / _DEG_HIST.sum())


def _ceil_to(x, m):
    return ((x + m - 1) // m) * m


# --------------------------------------------------------------------------
# Host-side planning (sharding + layout)
# --------------------------------------------------------------------------

class Plan:
    pass


def make_plan(src, dst, n_nodes, n_cores=N_CORES):
    assert n_nodes % n_cores == 0
    npc = n_nodes // n_cores
    p = Plan()
    p.n_nodes = n_nodes
    p.npc = npc
    p.n_cores = n_cores

    owner = dst // npc
    core_edges = []
    core_deg = []
    dmax = 0
    for c in range(n_cores):
        sel = np.nonzero(owner == c)[0]
        dloc = dst[sel] - c * npc
        deg = np.bincount(dloc, minlength=npc)
        dmax = max(dmax, int(deg.max()) if deg.size else 0)
        core_edges.append(sel)
        core_deg.append(deg)
    assert dmax <= 512, f"degree {dmax} too large"
    p.dmax = dmax

    n_d_max = np.zeros(dmax + 1, dtype=np.int64)
    for c in range(n_cores):
        cnt = np.bincount(core_deg[c], minlength=dmax + 1)
        n_d_max = np.maximum(n_d_max, cnt)

    sec_size = [int(n_d_max[0])] + [int(n_d_max[d]) for d in range(1, dmax + 1)]
    sec_off = np.concatenate([[0], np.cumsum(sec_size)])
    n_used = int(sec_off[-1])
    p.N_layout = _ceil_to(max(n_used, 512), 512)
    p.n0_max = int(n_d_max[0])
    p.n_used = n_used

    chunks = []  # (d, g, cols, slot_base, node_base)
    sbase = 0
    for d in range(1, dmax + 1):
        rem = int(n_d_max[d])
        nbase = int(sec_off[d])
        gmax = 512 // d
        while rem > 0:
            g = min(rem, gmax)
            cols = _ceil_to(g * d, 128)
            chunks.append((d, g, cols, sbase, nbase))
            sbase += cols
            nbase += g
            rem -= g
    p.chunks = chunks
    p.S = sbase if sbase > 0 else 128

    # sections: per degree-class node range (d >= 1, nonempty)
    p.sections = []
    for d in range(1, dmax + 1):
        if n_d_max[d] > 0:
            p.sections.append((d, int(sec_off[d]), int(sec_off[d + 1])))
    p.dclasses = [d for (d, a, b) in p.sections]

    # gather groups of consecutive chunks, total cols <= GROUP_COLS
    groups = []
    cur = None
    for ci, (d, g, cols, sb, nb) in enumerate(chunks):
        if cur is None or cur[1] + cols > GROUP_COLS:
            cur = [sb, cols, [ci]]
            groups.append(cur)
        else:
            cur[1] += cols
            cur[2].append(ci)
    p.groups = [tuple(x) for x in groups]

    # per-core node layout + slot->edge map (k-major within chunks)
    p.layout_nodes = []
    p.core_edges_sorted = []
    for c in range(n_cores):
        deg = core_deg[c]
        lay = np.full(p.N_layout, -1, dtype=np.int64)
        for d in range(0, dmax + 1):
            ids = np.nonzero(deg == d)[0]
            lay[sec_off[d]:sec_off[d] + ids.size] = ids
        p.layout_nodes.append(lay)

        sel = core_edges[c]
        dloc = dst[sel] - c * npc
        eorder = np.argsort(dloc, kind="stable")
        sel_sorted = sel[eorder]
        starts = np.zeros(npc + 1, dtype=np.int64)
        starts[1:] = np.cumsum(deg)

        slot_edge = np.full(p.S, -1, dtype=np.int64)
        for (d, g, cols, sb, nb) in chunks:
            nodes = lay[nb:nb + g]
            real = np.nonzero(nodes >= 0)[0]
            ed = np.full((g, d), -1, dtype=np.int64)
            if real.size:
                rn = nodes[real]
                em = starts[rn][:, None] + np.arange(d)[None, :]
                ed[real] = sel_sorted[em]
            # k-major: slot = k*g + j
            slot_edge[sb:sb + g * d] = ed.T.ravel()
        p.core_edges_sorted.append(slot_edge)

    # weight-blob index layout (structure only; values filled by make_weights)
    p.widx = dict(WXI=0, WXJ=1, W2BD=2, IDENT=3, W1X=4, W2P=5, WLINC=6)
    p.wd_idx = {}
    p.cv_idx = {0: 0}
    bi, ci = 7, 1
    for d in p.dclasses:
        p.wd_idx[d] = bi
        bi += 1 if d == 1 else 4
        p.cv_idx[d] = ci
        ci += 1
    p.n_wblocks = bi
    p.n_cvrow = 1 + ci
    return p


def make_core_inputs(p, c, atom_x, bond_x, src, W):
    npc = p.npc
    lay = p.layout_nodes[c]
    slot_edge = p.core_edges_sorted[c]
    S = p.S

    import ml_dtypes
    valid = slot_edge >= 0
    se = np.maximum(slot_edge, 0)
    xj_id = np.where(valid, src[se], 0)
    xj = atom_x[xj_id]
    xj[~valid] = 0.0
    xjT = np.ascontiguousarray(xj.T.astype(ml_dtypes.bfloat16))

    bondT = np.zeros((S, EC), dtype=ml_dtypes.bfloat16)
    bondT[valid] = bond_x[slot_edge[valid]].astype(ml_dtypes.bfloat16)
    bondT = np.ascontiguousarray(bondT.T)

    gid = np.where(lay >= 0, c * npc + lay, 0)
    xl = atom_x[gid]                                    # [NL, 128]
    x_layT = np.ascontiguousarray(xl.T.astype(ml_dtypes.bfloat16))
    x_lay = np.ascontiguousarray(
        xl.reshape(p.N_layout // 128, 128, H).transpose(1, 0, 2).reshape(128, -1))

    m = dict(xjT=xjT, bondT=bondT, x_layT=x_layT, x_lay=x_lay)
    m.update(W)
    return m


def make_weights(inp, p):
    """Host-side weight folding. Returns dict of shared DRAM inputs."""
    import ml_dtypes
    BF = ml_dtypes.bfloat16
    W_pre1, b_pre1 = np.asarray(inp["W_pre1"], np.float64), np.asarray(inp["b_pre1"], np.float64)
    W_pre2, b_pre2 = np.asarray(inp["W_pre2"], np.float64), np.asarray(inp["b_pre2"], np.float64)
    W_post1, b_post1 = np.asarray(inp["W_post1"], np.float64), np.asarray(inp["b_post1"], np.float64)
    W_post2, b_post2 = np.asarray(inp["W_post2"], np.float64), np.asarray(inp["b_post2"], np.float64)
    W_lin, b_lin = np.asarray(inp["W_lin"], np.float64), np.asarray(inp["b_lin"], np.float64)
    W_bond, b_bond = np.asarray(inp["W_bond"], np.float64), np.asarray(inp["b_bond"], np.float64)
    W_ee, b_ee = np.asarray(inp["W_ee"], np.float64), np.asarray(inp["b_ee"], np.float64)
    assert np.allclose(np.asarray(inp["ln_w"]), 1.0) and \
        np.allclose(np.asarray(inp["ln_b"]), 0.0), "ln affine not identity"

    def blockdiag(mats):
        n = len(mats)
        r, co = mats[0].shape
        out = np.zeros((n * r, n * co), dtype=np.float64)
        for t in range(n):
            out[t * r:(t + 1) * r, t * co:(t + 1) * co] = mats[t]
        return out

    W_be = W_bond @ W_ee
    b_be = b_bond @ W_ee + b_ee
    Wxi = blockdiag([W_pre1[t][0:F_IN] for t in range(T)])
    Wxj = blockdiag([W_pre1[t][F_IN:2 * F_IN] for t in range(T)])
    Wea_cat = np.concatenate([W_pre1[t][2 * F_IN:3 * F_IN] for t in range(T)], axis=1)
    W_bea = W_be @ Wea_cat
    b1p = b_pre1.reshape(H) + b_be @ Wea_cat
    W2bd = blockdiag([W_pre2[t] for t in range(T)])
    b2 = b_pre2.reshape(H)
    W1X = blockdiag([W_post1[t][0:F_IN] for t in range(T)])
    W1 = [[blockdiag([W_post1[t][F_IN + r * 5 * F_IN + a * F_IN:
                                 F_IN + r * 5 * F_IN + (a + 1) * F_IN]
                      for t in range(T)]) for a in range(5)] for r in range(3)]
    W2p = blockdiag([W_post2[t] for t in range(T)])
    b3 = b_post2.reshape(H)
    bp1 = b_post1.reshape(H)

    # LN centering fold
    P_c = np.eye(H) - np.ones((H, H)) / H
    WLINc = W_lin @ P_c
    blinc = P_c @ b_lin

    # per-degree-class combined weights + corrections
    wblocks = [Wxi, Wxj, W2bd, np.eye(H), W1X, W2p, WLINc]
    cvecs = []

    def wcomb(d):
        dc = max(d, 1.0)
        logdeg = np.log(dc + 1.0)
        amp, att = logdeg / AVG_DEG_LOG, AVG_DEG_LOG / logdeg
        return [W1[0][a] + amp * W1[1][a] + att * W1[2][a] for a in range(5)]

    # class 0 = degree 0: std = sqrt(eps), everything else zero
    Wc0 = wcomb(0)
    cvecs.append(np.sqrt(EPS) * Wc0[4].sum(axis=0))
    for d in p.dclasses:
        Wc = wcomb(d)
        if d == 1:
            wblocks.append(Wc[0] + Wc[1] + Wc[2] + Wc[3])
            cvecs.append(b2 @ (1 * Wc[0] + Wc[1] + Wc[2] + Wc[3])
                         + np.sqrt(EPS) * Wc[4].sum(axis=0))
        else:
            wblocks.extend([Wc[0] + Wc[1] / d, Wc[2], Wc[3], Wc[4]])
            cvecs.append(b2 @ (d * Wc[0] + Wc[1] + Wc[2] + Wc[3]))

    assert len(wblocks) == p.n_wblocks
    wmm = np.concatenate([np.asarray(w) for w in wblocks], axis=1).astype(BF)
    cvrow = np.concatenate([np.ones(H)] + cvecs).astype(BF)[None, :]
    assert cvrow.shape[1] == p.n_cvrow * 128, (cvrow.shape, p.n_cvrow)
    baux = np.stack([b1p, bp1, b3, blinc], axis=1).astype(np.float32)  # [128,4]
    return dict(
        wmm=np.ascontiguousarray(wmm),
        wbea16=np.ascontiguousarray(W_bea.astype(BF)),
        cvrow=np.ascontiguousarray(cvrow),
        baux=np.ascontiguousarray(baux),
    )


# --------------------------------------------------------------------------
# Bass kernel builder
# --------------------------------------------------------------------------

def build_nc(p, n_nodes, debug=False):
    nc = bacc.Bacc("TRN2", target_bir_lowering=False, debug=debug)
    S, NL = p.S, p.N_layout
    NB = NL // 512

    xjT_d = nc.dram_tensor("xjT", [128, S], BF16, kind="ExternalInput")
    bondT_d = nc.dram_tensor("bondT", [EC, S], BF16, kind="ExternalInput")
    xlt_d = nc.dram_tensor("x_layT", [128, NL], BF16, kind="ExternalInput")
    xln_d = nc.dram_tensor("x_lay", [128, NL], F32, kind="ExternalInput")
    wmm_d = nc.dram_tensor("wmm", [128, p.n_wblocks * 128], BF16, kind="ExternalInput")
    wbea_d = nc.dram_tensor("wbea16", [EC, 128], BF16, kind="ExternalInput")
    cvrow_d = nc.dram_tensor("cvrow", [1, p.n_cvrow * 128], BF16, kind="ExternalInput")
    baux_d = nc.dram_tensor("baux", [128, 4], F32, kind="ExternalInput")
    out_d = nc.dram_tensor("out", [NL, H], F32, kind="ExternalOutput")

    # blocks -> pieces; piece = (d, a, b) with [a,b) within block
    sec_all = [(0, 0, p.n0_max)] + p.sections
    block_pieces = [[] for _ in range(NB)]
    for (d, a, b) in sec_all:
        if b <= a:
            continue
        t0, t1 = a // 512, (b - 1) // 512
        for t in range(t0, t1 + 1):
            pa, pb_ = max(a, t * 512), min(b, (t + 1) * 512)
            if pb_ > pa:
                block_pieces[t].append((d, pa, pb_))

    # block -> last chunk index contributing to it
    last_chunk = [-1] * NB
    for ci, (d, g, cols, sb, nb) in enumerate(p.chunks):
        for t in range(nb // 512, min((nb + g - 1) // 512, NB - 1) + 1):
            last_chunk[t] = max(last_chunk[t], ci)
    # ensure monotone (a block can't be ready before an earlier block's chunks)
    for t in range(1, NB):
        last_chunk[t] = max(last_chunk[t], last_chunk[t - 1])

    with tile.TileContext(nc) as tc:
        from contextlib import ExitStack
        with ExitStack() as ctx:
            cpool = ctx.enter_context(tc.tile_pool(name="consts", bufs=1))
            wmm = cpool.tile([128, p.n_wblocks * 128], BF16)
            nc.sync.dma_start(wmm[:], wmm_d[:])
            wbea16 = cpool.tile([EC, 128], BF16)
            nc.sync.dma_start(wbea16[:], wbea_d[:])
            cvrow = cpool.tile([1, p.n_cvrow * 128], BF16)
            nc.sync.dma_start(cvrow[:], cvrow_d[:])
            baux = cpool.tile([128, 4], F32)
            nc.sync.dma_start(baux[:], baux_d[:])
            ident = cpool.tile([128, 128], F32)
            make_identity(nc, ident[:])
            epsc = cpool.tile([128, 1], F32)
            nc.vector.memset(epsc[:], EPS)
            onescol = cpool.tile([128, 1], BF16)
            nc.vector.memset(onescol[:], 1.0)
            ones512 = cpool.tile([1, 512], BF16)
            nc.vector.memset(ones512[:], 1.0)

            def WB(i):
                return wmm[:, i * 128:(i + 1) * 128]

            def CV(di):
                i = p.cv_idx[di] + 1  # +1: row block 0 is the ones row
                return cvrow[:, i * 128:(i + 1) * 128]

            ONESROW = cvrow[:, 0:128]
            B1P, BP1, B3, BLINC = (baux[:, i:i + 1] for i in range(4))

            # persistent aggregate arrays
            aggp = ctx.enter_context(tc.tile_pool(name="agg", bufs=1))
            s16_agg = aggp.tile([128, NL], BF16)
            mn16_agg = aggp.tile([128, NL], BF16)
            mx16_agg = aggp.tile([128, NL], BF16)
            std16_agg = aggp.tile([128, NL], BF16)

            ep = ctx.enter_context(tc.tile_pool(name="edge_sb", bufs=3))
            sp2 = ctx.enter_context(tc.tile_pool(name="stdchain_sb", bufs=2))
            gp = ctx.enter_context(tc.tile_pool(name="edge_gath", bufs=3))
            pb = ctx.enter_context(tc.tile_pool(name="post_sb", bufs=2))
            epp = ctx.enter_context(tc.tile_pool(name="edge_ps", bufs=3, space="PSUM"))
            sqp = ctx.enter_context(tc.tile_pool(name="sq_ps", bufs=2, space="PSUM"))
            opp = ctx.enter_context(tc.tile_pool(name="op_ps", bufs=2, space="PSUM"))
            mpp = ctx.enter_context(tc.tile_pool(name="misc_ps", bufs=1, space="PSUM"))

            gtiles = {}

            def emit_gather(gi):
                gsb, gcols, _ = p.groups[gi]
                xj_fmg = gp.tile([128, gcols], BF16, tag="xj_fmg")
                gtiles[gi] = xj_fmg
                nc.sync.dma_start(xj_fmg[:], xjT_d[:, gsb:gsb + gcols])

            def emit_chunk(gi, ci):
                gsb, gcols, _ = p.groups[gi]
                xj_fmg = gtiles[gi]
                d, g, cols, sb, nb = p.chunks[ci]
                loc = sb - gsb
                gd = g * d
                nsl = slice(nb, nb + g)

                bond_t = ep.tile([EC, 512], BF16, tag="bond")
                nc.sync.dma_start(bond_t[:, 0:gd], bondT_d[:, sb:sb + gd])
                xi_t = ep.tile([128, 512], BF16, tag="xi_t")
                nc.sync.dma_start(xi_t[:, 0:g], xlt_d[:, nsl])

                p1 = epp.tile([128, 512], F32, tag="mm")
                nc.tensor.matmul(out=p1[:, 0:gd], lhsT=wbea16[:],
                                 rhs=bond_t[:, 0:gd], start=True, stop=False)
                nc.tensor.matmul(out=p1[:, 0:gd], lhsT=WB(p.widx["WXJ"]),
                                 rhs=xj_fmg[:, loc:loc + gd], start=False, stop=False)
                nc.tensor.matmul(
                    out=p1[:, 0:gd].rearrange("p (k j) -> p k j", k=d),
                    lhsT=WB(p.widx["WXI"]),
                    rhs=xi_t[:, None, 0:g].to_broadcast((128, d, g)),
                    start=False, stop=True)
                h1 = ep.tile([128, 512], BF16, tag="h1")
                nc.scalar.activation(h1[:, 0:gd], p1[:, 0:gd], AF.Relu, bias=B1P)
                p2 = epp.tile([128, 512], F32, tag="mm")
                nc.tensor.matmul(out=p2[:, 0:gd], lhsT=WB(p.widx["W2BD"]),
                                 rhs=h1[:, 0:gd], start=True, stop=True)

                if d == 1:
                    nc.scalar.activation(s16_agg[:, nsl], p2[:, 0:g], AF.Copy)
                    return

                msq = ep.tile([128, 512], BF16, tag="msq")
                nc.scalar.activation(msq[:, 0:gd], p2[:, 0:gd], AF.Square)

                sq = sqp.tile([128, 512], F32, tag="sq")
                nc.tensor.matmul(
                    out=sq[:, None, 0:g].to_broadcast((128, d, g)),
                    lhsT=WB(p.widx["W2BD"]),
                    rhs=h1[:, 0:gd].rearrange("p (k j) -> p k j", k=d),
                    start=True, stop=True, skip_group_check=True)
                qt = sp2.tile([128, 256], F32, tag="qt")
                nc.vector.reduce_sum(
                    out=qt[:, 0:g],
                    in_=msq[:, 0:gd].rearrange("p (k j) -> p j k", k=d),
                    axis=AX.X)

                nc.scalar.activation(s16_agg[:, nsl], sq[:, 0:g], AF.Copy)
                msqt_t = sp2.tile([128, 256], F32, tag="msqt")
                nc.scalar.activation(msqt_t[:, 0:g], sq[:, 0:g], AF.Square,
                                     scale=1.0 / d)
                e2t = sp2.tile([128, 256], F32, tag="e2t")
                nc.scalar.activation(e2t[:, 0:g], qt[:, 0:g], AF.Copy,
                                     scale=1.0 / d)
                vt = sp2.tile([128, 256], F32, tag="vt")
                nc.gpsimd.tensor_tensor(vt[:, 0:g], e2t[:, 0:g], msqt_t[:, 0:g],
                                        OP.subtract)
                vt2 = sp2.tile([128, 256], F32, tag="vt2")
                # (var max 0) + eps in one DVE op; then a plain sqrt
                nc.vector.tensor_scalar(vt2[:, 0:g], vt[:, 0:g], 0.0, EPS,
                                        OP.max, OP.add)
                nc.scalar.activation(std16_agg[:, nsl], vt2[:, 0:g], AF.Sqrt)

                p2v = p2[:, 0:gd].rearrange("p (k j) -> p j k", k=d)
                nc.vector.tensor_reduce(out=mn16_agg[:, nsl], in_=p2v,
                                        axis=AX.X, op=OP.min)
                nc.vector.tensor_reduce(out=mx16_agg[:, nsl], in_=p2v,
                                        axis=AX.X, op=OP.max)

            def emit_block(t):
                nb = t * 512
                nsl = slice(nb, nb + 512)
                pieces = block_pieces[t]

                x_fm = pb.tile([128, 512], BF16, tag="x_fm")
                nc.sync.dma_start(x_fm[:], xlt_d[:, nsl])
                x_nm = pb.tile([128, 512], F32, tag="x_nm")
                nc.sync.dma_start(x_nm[:], xln_d[:, nsl])

                op = opp.tile([128, 512], F32, tag="op")
                mms = [dict(out=op[:], lhsT=WB(p.widx["W1X"]), rhs=x_fm[:])]
                for (d, a, b) in pieces:
                    rel = slice(a - nb, b - nb)
                    n = b - a
                    if d == 0:
                        mms.append(dict(out=op[:, rel], lhsT=CV(0),
                                        rhs=ones512[:, 0:n]))
                    elif d == 1:
                        wi = p.wd_idx[1]
                        mms.append(dict(out=op[:, rel], lhsT=WB(wi),
                                        rhs=s16_agg[:, a:b]))
                        mms.append(dict(out=op[:, rel], lhsT=CV(1),
                                        rhs=ones512[:, 0:n]))
                    else:
                        wi = p.wd_idx[d]
                        mms.append(dict(out=op[:, rel], lhsT=WB(wi),
                                        rhs=s16_agg[:, a:b]))
                        mms.append(dict(out=op[:, rel], lhsT=WB(wi + 1),
                                        rhs=mn16_agg[:, a:b]))
                        mms.append(dict(out=op[:, rel], lhsT=WB(wi + 2),
                                        rhs=mx16_agg[:, a:b]))
                        mms.append(dict(out=op[:, rel], lhsT=WB(wi + 3),
                                        rhs=std16_agg[:, a:b]))
                        mms.append(dict(out=op[:, rel], lhsT=CV(d),
                                        rhs=ones512[:, 0:n]))
                for i, kw in enumerate(mms):
                    nc.tensor.matmul(start=(i == 0), stop=(i == len(mms) - 1),
                                     skip_group_check=True, **kw)

                h1p = pb.tile([128, 512], BF16, tag="h1p")
                nc.vector.tensor_scalar(h1p[:], op[:], BP1, 0.0, OP.add, OP.max)
                pp2 = mpp.tile([128, 512], F32, tag="mp")
                nc.tensor.matmul(out=pp2[:], lhsT=WB(p.widx["W2P"]), rhs=h1p[:],
                                 start=True, stop=True)
                z2 = pb.tile([128, 512], BF16, tag="z2")
                nc.vector.tensor_scalar(z2[:], pp2[:], B3, None, OP.add)
                plin = mpp.tile([128, 512], F32, tag="mp")
                nc.tensor.matmul(out=plin[:], lhsT=WB(p.widx["WLINC"]), rhs=z2[:],
                                 start=True, stop=True)
                zf = pb.tile([128, 512], F32, tag="zf")
                nc.scalar.activation(zf[:], plin[:], AF.Identity, bias=BLINC)
                sq16 = pb.tile([128, 512], BF16, tag="sq16")
                nc.gpsimd.tensor_tensor(sq16[:], zf[:], zf[:], OP.mult)
                vs = mpp.tile([128, 512], F32, tag="mp")
                nc.tensor.matmul(out=vs[0:1, :], lhsT=onescol[:], rhs=sq16[:],
                                 start=True, stop=True)
                sd = pb.tile([1, 512], F32, tag="sd")
                nc.scalar.activation(sd[:], vs[0:1, :], AF.Sqrt,
                                     scale=1.0 / 128.0, bias=epsc[0:1, :])
                ri32 = pb.tile([1, 512], F32, tag="ri32")
                nc.vector.reciprocal_approx_fast(ri32[:], sd[:])
                ri = pb.tile([1, 512], BF16, tag="ri")
                with nc.allow_low_precision(reason="rstd broadcast via bf16 ones matmul"):
                    nc.vector.tensor_copy(ri[:], ri32[:])
                rb = mpp.tile([128, 512], F32, tag="mp")
                nc.tensor.matmul(out=rb[:], lhsT=ONESROW, rhs=ri[:],
                                 start=True, stop=True)
                y = pb.tile([128, 512], F32, tag="y")
                nc.vector.tensor_tensor(y[:], zf[:], rb[:], OP.mult)
                ry = pb.tile([128, 512], F32, tag="ry")
                nc.vector.tensor_scalar(ry[:], y[:], 0.0, None, OP.max)
                zps = mpp.tile([128, 512], F32, tag="mp")
                for bq in range(4):
                    sl = slice(128 * bq, 128 * (bq + 1))
                    nc.tensor.transpose(out=zps[:, sl], in_=ry[:, sl],
                                        identity=ident[:])
                out_nm = pb.tile([128, 512], F32, tag="out_nm")
                nc.vector.tensor_tensor(out_nm[:], zps[:], x_nm[:], OP.add)
                nc.sync.dma_start(
                    out_d[nsl, :].rearrange("(j p) f -> p j f", p=128),
                    out_nm[:].rearrange("p (j f) -> p j f", f=128))

            # interleaved emission
            next_block = 0
            for gi in range(len(p.groups)):
                emit_gather(gi)
                for ci in p.groups[gi][2]:
                    emit_chunk(gi, ci)
                    while next_block < NB and last_chunk[next_block] <= ci:
                        emit_block(next_block)
                        next_block += 1
            while next_block < NB:
                emit_block(next_block)
                next_block += 1
    nc.compile()
    return nc


# --------------------------------------------------------------------------
# Entry point
# --------------------------------------------------------------------------

_CACHE = {}


def _get_compiled(src, dst, n_nodes):
    key = hash((src.tobytes(), dst.tobytes(), n_nodes))
    if key not in _CACHE:
        p = make_plan(src.astype(np.int64), dst.astype(np.int64), n_nodes)
        nc = build_nc(p, n_nodes)
        _CACHE[key] = (p, nc)
    return _CACHE[key]


def kernel(**inputs):
    atom_x = np.asarray(inputs["atom_x"], np.float32)
    bond_x = np.asarray(inputs["bond_x"], np.float32)
    ei = np.asarray(inputs["atom_edge_index"])
    src = ei[0].astype(np.int64)
    n_nodes = atom_x.shape[0]

    p, nc = _get_compiled(ei[0], ei[1], n_nodes)
    W = make_weights(inputs, p)
    in_maps = [make_core_inputs(p, c, atom_x, bond_x, src, W)
               for c in range(p.n_cores)]
    res = run_bass_kernel_spmd(nc, in_maps, core_ids=list(range(p.n_cores)))

    out = np.zeros((n_nodes, H), dtype=np.float32)
    for c in range(p.n_cores):
        o = res.results[c]["out"]
        lay = p.layout_nodes[c]
        real = np.nonzero(lay >= 0)[0]
        out[c * p.npc + lay[real]] = o[real]
    return out


# revision 28
# speedup vs baseline: 1.4565x; 1.4565x over previous
"""Trainium2 Bass kernel for nn_Drug_PNAConv (GNN message passing, PNAConv).

v3c strategy:
  - Nodes partitioned by destination across 8 cores; host bins edges by dest
    degree into fixed chunks (g nodes x degree d), slots k-major ([d, g]).
  - Source features pre-gathered ON HOST into a contiguous feature-major bf16
    stream xjT [128, S]; no device gather at all.
  - Edge phase per chunk: p1 = Wbea.bond + Wxj.xj + Wxi.xi (PSUM), h1 = relu
    (scalar evac), p2 = W2BD.h1.  Segmented SUM and SUMSQ via single matmuls
    with stride-0 output APs (PSUM self-accumulation); min/max via DVE
    tensor_reduce directly from p2 PSUM (raw, pre-bias).
  - b_pre2 never applied per-edge: sum/mean/min/max offsets are folded into
    per-degree-class correction vectors (rank-1 ones matmuls); variance is
    shift-invariant so the std chain uses raw moments.
  - Degree scalers (identity/amplification/attenuation) folded into
    per-degree-class combined post weights W(d,a) = W1(0,a) + amp_d*W1(1,a)
    + att_d*W1(2,a); mean folded into the s-weight (Wsm = Wc0 + Wc1/d).
    d=0 and d=1 collapse further (std=sqrt(eps) exactly).
  - LayerNorm mean-centering folded into W_lin on host (P_c = I - 11^T/128);
    variance via ones-column matmul; rstd broadcast via ones-row matmul.
  - Edge and post phases interleaved per 512-node block for engine overlap.
"""

import os
import sys

for _p in ("/opt/trn_rl_repo", os.path.expanduser("~/.axon_site/_ro/trn_rl_repo")):
    if os.path.isdir(_p) and _p not in sys.path:
        sys.path.insert(0, _p)

import numpy as np

import concourse.bass as bass
import concourse.bacc as bacc
import concourse.mybir as mybir
import concourse.tile as tile
from concourse.bass_utils import run_bass_kernel_spmd
from concourse.masks import make_identity

F32 = mybir.dt.float32
BF16 = mybir.dt.bfloat16
AF = mybir.ActivationFunctionType
OP = mybir.AluOpType
AX = mybir.AxisListType

N_CORES = 8
H = 128
T = 4
F_IN = 32
EC = 16
EPS = 1e-5
GROUP_COLS = 2048

_DEG_HIST = np.array([0.0, 5000.0, 20000.0, 25000.0, 10000.0])
_BINS = np.arange(_DEG_HIST.size)
AVG_DEG_LOG = float((np.log(_BINS + 1.0) * _DEG_HIST).sum() / _DEG_HIST.sum())


def _ceil_to(x, m):
    return ((x + m - 1) // m) * m


# --------------------------------------------------------------------------
# Host-side planning (sharding + layout)
# --------------------------------------------------------------------------

class Plan:
    pass


def make_plan(src, dst, n_nodes, n_cores=N_CORES):
    assert n_nodes % n_cores == 0
    npc = n_nodes // n_cores
    p = Plan()
    p.n_nodes = n_nodes
    p.npc = npc
    p.n_cores = n_cores

    owner = dst // npc
    core_edges = []
    core_deg = []
    dmax = 0
    for c in range(n_cores):
        sel = np.nonzero(owner == c)[0]
        dloc = dst[sel] - c * npc
        deg = np.bincount(dloc, minlength=npc)
        dmax = max(dmax, int(deg.max()) if deg.size else 0)
        core_edges.append(sel)
        core_deg.append(deg)
    assert dmax <= 512, f"degree {dmax} too large"
    p.dmax = dmax

    n_d_max = np.zeros(dmax + 1, dtype=np.int64)
    for c in range(n_cores):
        cnt = np.bincount(core_deg[c], minlength=dmax + 1)
        n_d_max = np.maximum(n_d_max, cnt)

    sec_size = [int(n_d_max[0])] + [int(n_d_max[d]) for d in range(1, dmax + 1)]
    sec_off = np.concatenate([[0], np.cumsum(sec_size)])
    n_used = int(sec_off[-1])
    p.N_layout = _ceil_to(max(n_used, 512), 512)
    p.n0_max = int(n_d_max[0])
    p.n_used = n_used

    chunks = []  # (d, g, cols, slot_base, node_base)
    sbase = 0
    for d in range(1, dmax + 1):
        rem = int(n_d_max[d])
        nbase = int(sec_off[d])
        gmax = 512 // d
        while rem > 0:
            g = min(rem, gmax)
            cols = _ceil_to(g * d, 128)
            chunks.append((d, g, cols, sbase, nbase))
            sbase += cols
            nbase += g
            rem -= g
    p.chunks = chunks
    p.S = sbase if sbase > 0 else 128

    # sections: per degree-class node range (d >= 1, nonempty)
    p.sections = []
    for d in range(1, dmax + 1):
        if n_d_max[d] > 0:
            p.sections.append((d, int(sec_off[d]), int(sec_off[d + 1])))
    p.dclasses = [d for (d, a, b) in p.sections]

    # gather groups of consecutive chunks, total cols <= GROUP_COLS
    groups = []
    cur = None
    for ci, (d, g, cols, sb, nb) in enumerate(chunks):
        if cur is None or cur[1] + cols > GROUP_COLS:
            cur = [sb, cols, [ci]]
            groups.append(cur)
        else:
            cur[1] += cols
            cur[2].append(ci)
    p.groups = [tuple(x) for x in groups]

    # per-core node layout + slot->edge map (k-major within chunks)
    p.layout_nodes = []
    p.core_edges_sorted = []
    for c in range(n_cores):
        deg = core_deg[c]
        lay = np.full(p.N_layout, -1, dtype=np.int64)
        for d in range(0, dmax + 1):
            ids = np.nonzero(deg == d)[0]
            lay[sec_off[d]:sec_off[d] + ids.size] = ids
        p.layout_nodes.append(lay)

        sel = core_edges[c]
        dloc = dst[sel] - c * npc
        eorder = np.argsort(dloc, kind="stable")
        sel_sorted = sel[eorder]
        starts = np.zeros(npc + 1, dtype=np.int64)
        starts[1:] = np.cumsum(deg)

        slot_edge = np.full(p.S, -1, dtype=np.int64)
        for (d, g, cols, sb, nb) in chunks:
            nodes = lay[nb:nb + g]
            real = np.nonzero(nodes >= 0)[0]
            ed = np.full((g, d), -1, dtype=np.int64)
            if real.size:
                rn = nodes[real]
                em = starts[rn][:, None] + np.arange(d)[None, :]
                ed[real] = sel_sorted[em]
            # k-major: slot = k*g + j
            slot_edge[sb:sb + g * d] = ed.T.ravel()
        p.core_edges_sorted.append(slot_edge)

    # weight-blob index layout (structure only; values filled by make_weights)
    p.widx = dict(WXI=0, WXJ=1, W2BD=2, IDENT=3, W1X=4, W2P=5, WLINC=6)
    p.wd_idx = {}
    p.cv_idx = {0: 0}
    bi, ci = 7, 1
    for d in p.dclasses:
        p.wd_idx[d] = bi
        bi += 1 if d == 1 else 4
        p.cv_idx[d] = ci
        ci += 1
    p.n_wblocks = bi
    p.n_cvrow = 1 + ci
    return p


def make_core_inputs(p, c, atom_x, bond_x, src, W):
    npc = p.npc
    lay = p.layout_nodes[c]
    slot_edge = p.core_edges_sorted[c]
    S = p.S

    import ml_dtypes
    valid = slot_edge >= 0
    se = np.maximum(slot_edge, 0)
    xj_id = np.where(valid, src[se], 0)
    xj = atom_x[xj_id]
    xj[~valid] = 0.0
    xjT = np.ascontiguousarray(xj.T.astype(ml_dtypes.bfloat16))

    bondT = np.zeros((S, EC), dtype=ml_dtypes.bfloat16)
    bondT[valid] = bond_x[slot_edge[valid]].astype(ml_dtypes.bfloat16)
    bondT = np.ascontiguousarray(bondT.T)

    gid = np.where(lay >= 0, c * npc + lay, 0)
    xl = atom_x[gid]                                    # [NL, 128]
    x_layT = np.ascontiguousarray(xl.T.astype(ml_dtypes.bfloat16))
    x_lay = np.ascontiguousarray(
        xl.reshape(p.N_layout // 128, 128, H).transpose(1, 0, 2).reshape(128, -1))

    m = dict(xjT=xjT, bondT=bondT, x_layT=x_layT, x_lay=x_lay)
    m.update(W)
    return m


def make_weights(inp, p):
    """Host-side weight folding. Returns dict of shared DRAM inputs."""
    import ml_dtypes
    BF = ml_dtypes.bfloat16
    W_pre1, b_pre1 = np.asarray(inp["W_pre1"], np.float64), np.asarray(inp["b_pre1"], np.float64)
    W_pre2, b_pre2 = np.asarray(inp["W_pre2"], np.float64), np.asarray(inp["b_pre2"], np.float64)
    W_post1, b_post1 = np.asarray(inp["W_post1"], np.float64), np.asarray(inp["b_post1"], np.float64)
    W_post2, b_post2 = np.asarray(inp["W_post2"], np.float64), np.asarray(inp["b_post2"], np.float64)
    W_lin, b_lin = np.asarray(inp["W_lin"], np.float64), np.asarray(inp["b_lin"], np.float64)
    W_bond, b_bond = np.asarray(inp["W_bond"], np.float64), np.asarray(inp["b_bond"], np.float64)
    W_ee, b_ee = np.asarray(inp["W_ee"], np.float64), np.asarray(inp["b_ee"], np.float64)
    assert np.allclose(np.asarray(inp["ln_w"]), 1.0) and \
        np.allclose(np.asarray(inp["ln_b"]), 0.0), "ln affine not identity"

    def blockdiag(mats):
        n = len(mats)
        r, co = mats[0].shape
        out = np.zeros((n * r, n * co), dtype=np.float64)
        for t in range(n):
            out[t * r:(t + 1) * r, t * co:(t + 1) * co] = mats[t]
        return out

    W_be = W_bond @ W_ee
    b_be = b_bond @ W_ee + b_ee
    Wxi = blockdiag([W_pre1[t][0:F_IN] for t in range(T)])
    Wxj = blockdiag([W_pre1[t][F_IN:2 * F_IN] for t in range(T)])
    Wea_cat = np.concatenate([W_pre1[t][2 * F_IN:3 * F_IN] for t in range(T)], axis=1)
    W_bea = W_be @ Wea_cat
    b1p = b_pre1.reshape(H) + b_be @ Wea_cat
    W2bd = blockdiag([W_pre2[t] for t in range(T)])
    b2 = b_pre2.reshape(H)
    W1X = blockdiag([W_post1[t][0:F_IN] for t in range(T)])
    W1 = [[blockdiag([W_post1[t][F_IN + r * 5 * F_IN + a * F_IN:
                                 F_IN + r * 5 * F_IN + (a + 1) * F_IN]
                      for t in range(T)]) for a in range(5)] for r in range(3)]
    W2p = blockdiag([W_post2[t] for t in range(T)])
    b3 = b_post2.reshape(H)
    bp1 = b_post1.reshape(H)

    # LN centering fold
    P_c = np.eye(H) - np.ones((H, H)) / H
    WLINc = W_lin @ P_c
    blinc = P_c @ b_lin

    # per-degree-class combined weights + corrections
    wblocks = [Wxi, Wxj, W2bd, np.eye(H), W1X, W2p, WLINc]
    cvecs = []

    def wcomb(d):
        dc = max(d, 1.0)
        logdeg = np.log(dc + 1.0)
        amp, att = logdeg / AVG_DEG_LOG, AVG_DEG_LOG / logdeg
        return [W1[0][a] + amp * W1[1][a] + att * W1[2][a] for a in range(5)]

    # class 0 = degree 0: std = sqrt(eps), everything else zero
    Wc0 = wcomb(0)
    cvecs.append(np.sqrt(EPS) * Wc0[4].sum(axis=0))
    for d in p.dclasses:
        Wc = wcomb(d)
        if d == 1:
            wblocks.append(Wc[0] + Wc[1] + Wc[2] + Wc[3])
            cvecs.append(b2 @ (1 * Wc[0] + Wc[1] + Wc[2] + Wc[3])
                         + np.sqrt(EPS) * Wc[4].sum(axis=0))
        else:
            wblocks.extend([Wc[0] + Wc[1] / d, Wc[2], Wc[3], Wc[4]])
            cvecs.append(b2 @ (d * Wc[0] + Wc[1] + Wc[2] + Wc[3]))

    assert len(wblocks) == p.n_wblocks
    wmm = np.concatenate([np.asarray(w) for w in wblocks], axis=1).astype(BF)
    cvrow = np.concatenate([np.ones(H)] + cvecs).astype(BF)[None, :]
    assert cvrow.shape[1] == p.n_cvrow * 128, (cvrow.shape, p.n_cvrow)
    baux = np.stack([b1p, bp1, b3, blinc], axis=1).astype(np.float32)  # [128,4]
    return dict(
        wmm=np.ascontiguousarray(wmm),
        wbea16=np.ascontiguousarray(W_bea.astype(BF)),
        cvrow=np.ascontiguousarray(cvrow),
        baux=np.ascontiguousarray(baux),
    )


# --------------------------------------------------------------------------
# Bass kernel builder
# --------------------------------------------------------------------------

def build_nc(p, n_nodes, debug=False):
    nc = bacc.Bacc("TRN2", target_bir_lowering=False, debug=debug)
    S, NL = p.S, p.N_layout
    NB = NL // 512

    xjT_d = nc.dram_tensor("xjT", [128, S], BF16, kind="ExternalInput")
    bondT_d = nc.dram_tensor("bondT", [EC, S], BF16, kind="ExternalInput")
    xlt_d = nc.dram_tensor("x_layT", [128, NL], BF16, kind="ExternalInput")
    xln_d = nc.dram_tensor("x_lay", [128, NL], F32, kind="ExternalInput")
    wmm_d = nc.dram_tensor("wmm", [128, p.n_wblocks * 128], BF16, kind="ExternalInput")
    wbea_d = nc.dram_tensor("wbea16", [EC, 128], BF16, kind="ExternalInput")
    cvrow_d = nc.dram_tensor("cvrow", [1, p.n_cvrow * 128], BF16, kind="ExternalInput")
    baux_d = nc.dram_tensor("baux", [128, 4], F32, kind="ExternalInput")
    out_d = nc.dram_tensor("out", [NL, H], F32, kind="ExternalOutput")

    # blocks -> pieces; piece = (d, a, b) with [a,b) within block
    sec_all = [(0, 0, p.n0_max)] + p.sections
    block_pieces = [[] for _ in range(NB)]
    for (d, a, b) in sec_all:
        if b <= a:
            continue
        t0, t1 = a // 512, (b - 1) // 512
        for t in range(t0, t1 + 1):
            pa, pb_ = max(a, t * 512), min(b, (t + 1) * 512)
            if pb_ > pa:
                block_pieces[t].append((d, pa, pb_))

    # block -> last chunk index contributing to it
    last_chunk = [-1] * NB
    for ci, (d, g, cols, sb, nb) in enumerate(p.chunks):
        for t in range(nb // 512, min((nb + g - 1) // 512, NB - 1) + 1):
            last_chunk[t] = max(last_chunk[t], ci)
    # ensure monotone (a block can't be ready before an earlier block's chunks)
    for t in range(1, NB):
        last_chunk[t] = max(last_chunk[t], last_chunk[t - 1])

    with tile.TileContext(nc) as tc:
        from contextlib import ExitStack
        with ExitStack() as ctx:
            cpool = ctx.enter_context(tc.tile_pool(name="consts", bufs=1))
            wmm = cpool.tile([128, p.n_wblocks * 128], BF16)
            nc.sync.dma_start(wmm[:], wmm_d[:])
            wbea16 = cpool.tile([EC, 128], BF16)
            nc.sync.dma_start(wbea16[:], wbea_d[:])
            cvrow = cpool.tile([1, p.n_cvrow * 128], BF16)
            nc.sync.dma_start(cvrow[:], cvrow_d[:])
            baux = cpool.tile([128, 4], F32)
            nc.sync.dma_start(baux[:], baux_d[:])
            ident = cpool.tile([128, 128], F32)
            make_identity(nc, ident[:])
            epsc = cpool.tile([128, 1], F32)
            nc.vector.memset(epsc[:], EPS)
            onescol = cpool.tile([128, 1], BF16)
            nc.vector.memset(onescol[:], 1.0)
            ones512 = cpool.tile([1, 512], BF16)
            nc.vector.memset(ones512[:], 1.0)

            def WB(i):
                return wmm[:, i * 128:(i + 1) * 128]

            def CV(di):
                i = p.cv_idx[di] + 1  # +1: row block 0 is the ones row
                return cvrow[:, i * 128:(i + 1) * 128]

            ONESROW = cvrow[:, 0:128]
            B1P, BP1, B3, BLINC = (baux[:, i:i + 1] for i in range(4))

            # persistent aggregate arrays
            aggp = ctx.enter_context(tc.tile_pool(name="agg", bufs=1))
            s16_agg = aggp.tile([128, NL], BF16)
            mn16_agg = aggp.tile([128, NL], BF16)
            mx16_agg = aggp.tile([128, NL], BF16)
            std16_agg = aggp.tile([128, NL], BF16)

            ep = ctx.enter_context(tc.tile_pool(name="edge_sb", bufs=3))
            sp2 = ctx.enter_context(tc.tile_pool(name="stdchain_sb", bufs=2))
            gp = ctx.enter_context(tc.tile_pool(name="edge_gath", bufs=3))
            pb = ctx.enter_context(tc.tile_pool(name="post_sb", bufs=2))
            epp = ctx.enter_context(tc.tile_pool(name="edge_ps", bufs=3, space="PSUM"))
            sqp = ctx.enter_context(tc.tile_pool(name="sq_ps", bufs=2, space="PSUM"))
            opp = ctx.enter_context(tc.tile_pool(name="op_ps", bufs=2, space="PSUM"))
            mpp = ctx.enter_context(tc.tile_pool(name="misc_ps", bufs=1, space="PSUM"))

            gtiles = {}

            def emit_gather(gi):
                gsb, gcols, _ = p.groups[gi]
                xj_fmg = gp.tile([128, gcols], BF16, tag="xj_fmg")
                gtiles[gi] = xj_fmg
                nc.sync.dma_start(xj_fmg[:], xjT_d[:, gsb:gsb + gcols])

            def emit_chunk(gi, ci):
                gsb, gcols, _ = p.groups[gi]
                xj_fmg = gtiles[gi]
                d, g, cols, sb, nb = p.chunks[ci]
                loc = sb - gsb
                gd = g * d
                nsl = slice(nb, nb + g)

                bond_t = ep.tile([EC, 512], BF16, tag="bond")
                nc.sync.dma_start(bond_t[:, 0:gd], bondT_d[:, sb:sb + gd])
                xi_t = ep.tile([128, 512], BF16, tag="xi_t")
                nc.sync.dma_start(xi_t[:, 0:g], xlt_d[:, nsl])

                p1 = epp.tile([128, 512], F32, tag="mm")
                nc.tensor.matmul(out=p1[:, 0:gd], lhsT=wbea16[:],
                                 rhs=bond_t[:, 0:gd], start=True, stop=False)
                nc.tensor.matmul(out=p1[:, 0:gd], lhsT=WB(p.widx["WXJ"]),
                                 rhs=xj_fmg[:, loc:loc + gd], start=False, stop=False)
                nc.tensor.matmul(
                    out=p1[:, 0:gd].rearrange("p (k j) -> p k j", k=d),
                    lhsT=WB(p.widx["WXI"]),
                    rhs=xi_t[:, None, 0:g].to_broadcast((128, d, g)),
                    start=False, stop=True)
                h1 = ep.tile([128, 512], BF16, tag="h1")
                nc.scalar.activation(h1[:, 0:gd], p1[:, 0:gd], AF.Relu, bias=B1P)
                p2 = epp.tile([128, 512], F32, tag="mm")
                nc.tensor.matmul(out=p2[:, 0:gd], lhsT=WB(p.widx["W2BD"]),
                                 rhs=h1[:, 0:gd], start=True, stop=True)

                if d == 1:
                    nc.scalar.activation(s16_agg[:, nsl], p2[:, 0:g], AF.Copy)
                    return

                msq = ep.tile([128, 512], BF16, tag="msq")
                nc.scalar.activation(msq[:, 0:gd], p2[:, 0:gd], AF.Square)

                sq = sqp.tile([128, 512], F32, tag="sq")
                nc.tensor.matmul(
                    out=sq[:, None, 0:g].to_broadcast((128, d, g)),
                    lhsT=WB(p.widx["W2BD"]),
                    rhs=h1[:, 0:gd].rearrange("p (k j) -> p k j", k=d),
                    start=True, stop=True, skip_group_check=True)
                nc.tensor.matmul(
                    out=sq[:, None, 256:256 + g].to_broadcast((128, d, g)),
                    lhsT=WB(p.widx["IDENT"]),
                    rhs=msq[:, 0:gd].rearrange("p (k j) -> p k j", k=d),
                    start=True, stop=True, skip_group_check=True)

                nc.scalar.activation(s16_agg[:, nsl], sq[:, 0:g], AF.Copy)
                msqt_t = sp2.tile([128, 256], F32, tag="msqt")
                nc.scalar.activation(msqt_t[:, 0:g], sq[:, 0:g], AF.Square,
                                     scale=1.0 / d)
                e2t = sp2.tile([128, 256], F32, tag="e2t")
                nc.scalar.activation(e2t[:, 0:g], sq[:, 256:256 + g], AF.Copy,
                                     scale=1.0 / d)
                vt = sp2.tile([128, 256], F32, tag="vt")
                nc.gpsimd.tensor_tensor(vt[:, 0:g], e2t[:, 0:g], msqt_t[:, 0:g],
                                        OP.subtract)
                vt2 = sp2.tile([128, 256], F32, tag="vt2")
                # (var max 0) + eps in one DVE op; then a plain sqrt
                nc.vector.tensor_scalar(vt2[:, 0:g], vt[:, 0:g], 0.0, EPS,
                                        OP.max, OP.add)
                nc.scalar.activation(std16_agg[:, nsl], vt2[:, 0:g], AF.Sqrt)

                p2v = p2[:, 0:gd].rearrange("p (k j) -> p j k", k=d)
                nc.vector.tensor_reduce(out=mn16_agg[:, nsl], in_=p2v,
                                        axis=AX.X, op=OP.min)
                nc.vector.tensor_reduce(out=mx16_agg[:, nsl], in_=p2v,
                                        axis=AX.X, op=OP.max)

            def emit_block(t):
                nb = t * 512
                nsl = slice(nb, nb + 512)
                pieces = block_pieces[t]

                x_fm = pb.tile([128, 512], BF16, tag="x_fm")
                nc.sync.dma_start(x_fm[:], xlt_d[:, nsl])
                x_nm = pb.tile([128, 512], F32, tag="x_nm")
                nc.sync.dma_start(x_nm[:], xln_d[:, nsl])

                op = opp.tile([128, 512], F32, tag="op")
                mms = [dict(out=op[:], lhsT=WB(p.widx["W1X"]), rhs=x_fm[:])]
                for (d, a, b) in pieces:
                    rel = slice(a - nb, b - nb)
                    n = b - a
                    if d == 0:
                        mms.append(dict(out=op[:, rel], lhsT=CV(0),
                                        rhs=ones512[:, 0:n]))
                    elif d == 1:
                        wi = p.wd_idx[1]
                        mms.append(dict(out=op[:, rel], lhsT=WB(wi),
                                        rhs=s16_agg[:, a:b]))
                        mms.append(dict(out=op[:, rel], lhsT=CV(1),
                                        rhs=ones512[:, 0:n]))
                    else:
                        wi = p.wd_idx[d]
                        mms.append(dict(out=op[:, rel], lhsT=WB(wi),
                                        rhs=s16_agg[:, a:b]))
                        mms.append(dict(out=op[:, rel], lhsT=WB(wi + 1),
                                        rhs=mn16_agg[:, a:b]))
                        mms.append(dict(out=op[:, rel], lhsT=WB(wi + 2),
                                        rhs=mx16_agg[:, a:b]))
                        mms.append(dict(out=op[:, rel], lhsT=WB(wi + 3),
                                        rhs=std16_agg[:, a:b]))
                        mms.append(dict(out=op[:, rel], lhsT=CV(d),
                                        rhs=ones512[:, 0:n]))
                for i, kw in enumerate(mms):
                    nc.tensor.matmul(start=(i == 0), stop=(i == len(mms) - 1),
                                     skip_group_check=True, **kw)

                h1p = pb.tile([128, 512], BF16, tag="h1p")
                nc.vector.tensor_scalar(h1p[:], op[:], BP1, 0.0, OP.add, OP.max)
                pp2 = mpp.tile([128, 512], F32, tag="mp")
                nc.tensor.matmul(out=pp2[:], lhsT=WB(p.widx["W2P"]), rhs=h1p[:],
                                 start=True, stop=True)
                z2 = pb.tile([128, 512], BF16, tag="z2")
                nc.vector.tensor_scalar(z2[:], pp2[:], B3, None, OP.add)
                plin = mpp.tile([128, 512], F32, tag="mp")
                nc.tensor.matmul(out=plin[:], lhsT=WB(p.widx["WLINC"]), rhs=z2[:],
                                 start=True, stop=True)
                zf = pb.tile([128, 512], F32, tag="zf")
                nc.scalar.activation(zf[:], plin[:], AF.Identity, bias=BLINC)
                sq16 = pb.tile([128, 512], BF16, tag="sq16")
                nc.gpsimd.tensor_tensor(sq16[:], zf[:], zf[:], OP.mult)
                vs = mpp.tile([128, 512], F32, tag="mp")
                nc.tensor.matmul(out=vs[0:1, :], lhsT=onescol[:], rhs=sq16[:],
                                 start=True, stop=True)
                sd = pb.tile([1, 512], F32, tag="sd")
                nc.scalar.activation(sd[:], vs[0:1, :], AF.Sqrt,
                                     scale=1.0 / 128.0, bias=epsc[0:1, :])
                ri32 = pb.tile([1, 512], F32, tag="ri32")
                nc.vector.reciprocal_approx_fast(ri32[:], sd[:])
                ri = pb.tile([1, 512], BF16, tag="ri")
                with nc.allow_low_precision(reason="rstd broadcast via bf16 ones matmul"):
                    nc.vector.tensor_copy(ri[:], ri32[:])
                rb = mpp.tile([128, 512], F32, tag="mp")
                nc.tensor.matmul(out=rb[:], lhsT=ONESROW, rhs=ri[:],
                                 start=True, stop=True)
                y = pb.tile([128, 512], F32, tag="y")
                nc.vector.tensor_tensor(y[:], zf[:], rb[:], OP.mult)
                ry = pb.tile([128, 512], F32, tag="ry")
                nc.vector.tensor_scalar(ry[:], y[:], 0.0, None, OP.max)
                zps = mpp.tile([128, 512], F32, tag="mp")
                for bq in range(4):
                    sl = slice(128 * bq, 128 * (bq + 1))
                    nc.tensor.transpose(out=zps[:, sl], in_=ry[:, sl],
                                        identity=ident[:])
                out_nm = pb.tile([128, 512], F32, tag="out_nm")
                nc.vector.tensor_tensor(out_nm[:], zps[:], x_nm[:], OP.add)
                nc.sync.dma_start(
                    out_d[nsl, :].rearrange("(j p) f -> p j f", p=128),
                    out_nm[:].rearrange("p (j f) -> p j f", f=128))

            # interleaved emission
            next_block = 0
            for gi in range(len(p.groups)):
                emit_gather(gi)
                for ci in p.groups[gi][2]:
                    emit_chunk(gi, ci)
                    while next_block < NB and last_chunk[next_block] <= ci:
                        emit_block(next_block)
                        next_block += 1
            while next_block < NB:
                emit_block(next_block)
                next_block += 1
    nc.compile()
    return nc


# --------------------------------------------------------------------------
# Entry point
# --------------------------------------------------------------------------

_CACHE = {}


def _get_compiled(src, dst, n_nodes):
    key = hash((src.tobytes(), dst.tobytes(), n_nodes))
    if key not in _CACHE:
        p = make_plan(src.astype(np.int64), dst.astype(np.int64), n_nodes)
        nc = build_nc(p, n_nodes)
        _CACHE[key] = (p, nc)
    return _CACHE[key]


def kernel(**inputs):
    atom_x = np.asarray(inputs["atom_x"], np.float32)
    bond_x = np.asarray(inputs["bond_x"], np.float32)
    ei = np.asarray(inputs["atom_edge_index"])
    src = ei[0].astype(np.int64)
    n_nodes = atom_x.shape[0]

    p, nc = _get_compiled(ei[0], ei[1], n_nodes)
    W = make_weights(inputs, p)
    in_maps = [make_core_inputs(p, c, atom_x, bond_x, src, W)
               for c in range(p.n_cores)]
    res = run_bass_kernel_spmd(nc, in_maps, core_ids=list(range(p.n_cores)))

    out = np.zeros((n_nodes, H), dtype=np.float32)
    for c in range(p.n_cores):
        o = res.results[c]["out"]
        lay = p.layout_nodes[c]
        real = np.nonzero(lay >= 0)[0]
        out[c * p.npc + lay[real]] = o[real]
    return out


# revision 31
# speedup vs baseline: 1.5000x; 1.0299x over previous
"""Trainium2 Bass kernel for nn_Drug_PNAConv (GNN message passing, PNAConv).

v3c strategy:
  - Nodes partitioned by destination across 8 cores; host bins edges by dest
    degree into fixed chunks (g nodes x degree d), slots k-major ([d, g]).
  - Source features pre-gathered ON HOST into a contiguous feature-major bf16
    stream xjT [128, S]; no device gather at all.
  - Edge phase per chunk: p1 = Wbea.bond + Wxj.xj + Wxi.xi (PSUM), h1 = relu
    (scalar evac), p2 = W2BD.h1.  Segmented SUM and SUMSQ via single matmuls
    with stride-0 output APs (PSUM self-accumulation); min/max via DVE
    tensor_reduce directly from p2 PSUM (raw, pre-bias).
  - b_pre2 never applied per-edge: sum/mean/min/max offsets are folded into
    per-degree-class correction vectors (rank-1 ones matmuls); variance is
    shift-invariant so the std chain uses raw moments.
  - Degree scalers (identity/amplification/attenuation) folded into
    per-degree-class combined post weights W(d,a) = W1(0,a) + amp_d*W1(1,a)
    + att_d*W1(2,a); mean folded into the s-weight (Wsm = Wc0 + Wc1/d).
    d=0 and d=1 collapse further (std=sqrt(eps) exactly).
  - LayerNorm mean-centering folded into W_lin on host (P_c = I - 11^T/128);
    variance via ones-column matmul; rstd broadcast via ones-row matmul.
  - Edge and post phases interleaved per 512-node block for engine overlap.
"""

import os
import sys

for _p in ("/opt/trn_rl_repo", os.path.expanduser("~/.axon_site/_ro/trn_rl_repo")):
    if os.path.isdir(_p) and _p not in sys.path:
        sys.path.insert(0, _p)

import numpy as np

import concourse.bass as bass
import concourse.bacc as bacc
import concourse.mybir as mybir
import concourse.tile as tile
from concourse.bass_utils import run_bass_kernel_spmd
from concourse.masks import make_identity

F32 = mybir.dt.float32
BF16 = mybir.dt.bfloat16
AF = mybir.ActivationFunctionType
OP = mybir.AluOpType
AX = mybir.AxisListType

N_CORES = 8
H = 128
T = 4
F_IN = 32
EC = 16
EPS = 1e-5
GROUP_COLS = 2048

_DEG_HIST = np.array([0.0, 5000.0, 20000.0, 25000.0, 10000.0])
_BINS = np.arange(_DEG_HIST.size)
AVG_DEG_LOG = float((np.log(_BINS + 1.0) * _DEG_HIST).sum() / _DEG_HIST.sum())


def _ceil_to(x, m):
    return ((x + m - 1) // m) * m


# --------------------------------------------------------------------------
# Host-side planning (sharding + layout)
# --------------------------------------------------------------------------

class Plan:
    pass


def make_plan(src, dst, n_nodes, n_cores=N_CORES):
    assert n_nodes % n_cores == 0
    npc = n_nodes // n_cores
    p = Plan()
    p.n_nodes = n_nodes
    p.npc = npc
    p.n_cores = n_cores

    owner = dst // npc
    core_edges = []
    core_deg = []
    dmax = 0
    for c in range(n_cores):
        sel = np.nonzero(owner == c)[0]
        dloc = dst[sel] - c * npc
        deg = np.bincount(dloc, minlength=npc)
        dmax = max(dmax, int(deg.max()) if deg.size else 0)
        core_edges.append(sel)
        core_deg.append(deg)
    assert dmax <= 512, f"degree {dmax} too large"
    p.dmax = dmax

    n_d_max = np.zeros(dmax + 1, dtype=np.int64)
    for c in range(n_cores):
        cnt = np.bincount(core_deg[c], minlength=dmax + 1)
        n_d_max = np.maximum(n_d_max, cnt)

    sec_size = [int(n_d_max[0])] + [int(n_d_max[d]) for d in range(1, dmax + 1)]
    sec_off = np.concatenate([[0], np.cumsum(sec_size)])
    n_used = int(sec_off[-1])
    p.N_layout = _ceil_to(max(n_used, 512), 512)
    p.n0_max = int(n_d_max[0])
    p.n_used = n_used

    chunks = []  # (d, g, cols, slot_base, node_base)
    sbase = 0
    for d in range(1, dmax + 1):
        rem = int(n_d_max[d])
        nbase = int(sec_off[d])
        gmax = 512 // d
        while rem > 0:
            g = min(rem, gmax)
            cols = _ceil_to(g * d, 128)
            chunks.append((d, g, cols, sbase, nbase))
            sbase += cols
            nbase += g
            rem -= g
    p.chunks = chunks
    p.S = sbase if sbase > 0 else 128

    # sections: per degree-class node range (d >= 1, nonempty)
    p.sections = []
    for d in range(1, dmax + 1):
        if n_d_max[d] > 0:
            p.sections.append((d, int(sec_off[d]), int(sec_off[d + 1])))
    p.dclasses = [d for (d, a, b) in p.sections]

    # gather groups of consecutive chunks, total cols <= GROUP_COLS
    groups = []
    cur = None
    for ci, (d, g, cols, sb, nb) in enumerate(chunks):
        if cur is None or cur[1] + cols > GROUP_COLS:
            cur = [sb, cols, [ci]]
            groups.append(cur)
        else:
            cur[1] += cols
            cur[2].append(ci)
    p.groups = [tuple(x) for x in groups]

    # per-core node layout + slot->edge map (k-major within chunks)
    p.layout_nodes = []
    p.core_edges_sorted = []
    for c in range(n_cores):
        deg = core_deg[c]
        lay = np.full(p.N_layout, -1, dtype=np.int64)
        for d in range(0, dmax + 1):
            ids = np.nonzero(deg == d)[0]
            lay[sec_off[d]:sec_off[d] + ids.size] = ids
        p.layout_nodes.append(lay)

        sel = core_edges[c]
        dloc = dst[sel] - c * npc
        eorder = np.argsort(dloc, kind="stable")
        sel_sorted = sel[eorder]
        starts = np.zeros(npc + 1, dtype=np.int64)
        starts[1:] = np.cumsum(deg)

        slot_edge = np.full(p.S, -1, dtype=np.int64)
        for (d, g, cols, sb, nb) in chunks:
            nodes = lay[nb:nb + g]
            real = np.nonzero(nodes >= 0)[0]
            ed = np.full((g, d), -1, dtype=np.int64)
            if real.size:
                rn = nodes[real]
                em = starts[rn][:, None] + np.arange(d)[None, :]
                ed[real] = sel_sorted[em]
            # k-major: slot = k*g + j
            slot_edge[sb:sb + g * d] = ed.T.ravel()
        p.core_edges_sorted.append(slot_edge)

    # weight-blob index layout (structure only; values filled by make_weights)
    p.widx = dict(WXI=0, WXJ=1, W2BD=2, IDENT=3, W1X=4, W2P=5, WLINC=6)
    p.wd_idx = {}
    p.cv_idx = {0: 0}
    bi, ci = 7, 1
    for d in p.dclasses:
        p.wd_idx[d] = bi
        bi += 1 if d == 1 else 4
        p.cv_idx[d] = ci
        ci += 1
    p.n_wblocks = bi
    p.n_cvrow = 1 + ci
    return p


def make_core_inputs(p, c, atom_x, bond_x, src, W):
    npc = p.npc
    lay = p.layout_nodes[c]
    slot_edge = p.core_edges_sorted[c]
    S = p.S

    import ml_dtypes
    valid = slot_edge >= 0
    se = np.maximum(slot_edge, 0)
    xj_id = np.where(valid, src[se], 0)
    xj = atom_x[xj_id]
    xj[~valid] = 0.0
    xjT = np.ascontiguousarray(xj.T.astype(ml_dtypes.bfloat16))

    bondT = np.zeros((S, EC), dtype=ml_dtypes.bfloat16)
    bondT[valid] = bond_x[slot_edge[valid]].astype(ml_dtypes.bfloat16)
    bondT = np.ascontiguousarray(bondT.T)

    gid = np.where(lay >= 0, c * npc + lay, 0)
    xl = atom_x[gid]                                    # [NL, 128]
    x_layT = np.ascontiguousarray(xl.T.astype(ml_dtypes.bfloat16))
    x_lay = np.ascontiguousarray(
        xl.reshape(p.N_layout // 128, 128, H).transpose(1, 0, 2).reshape(128, -1))

    m = dict(xjT=xjT, bondT=bondT, x_layT=x_layT, x_lay=x_lay)
    m.update(W)
    return m


def make_weights(inp, p):
    """Host-side weight folding. Returns dict of shared DRAM inputs."""
    import ml_dtypes
    BF = ml_dtypes.bfloat16
    W_pre1, b_pre1 = np.asarray(inp["W_pre1"], np.float64), np.asarray(inp["b_pre1"], np.float64)
    W_pre2, b_pre2 = np.asarray(inp["W_pre2"], np.float64), np.asarray(inp["b_pre2"], np.float64)
    W_post1, b_post1 = np.asarray(inp["W_post1"], np.float64), np.asarray(inp["b_post1"], np.float64)
    W_post2, b_post2 = np.asarray(inp["W_post2"], np.float64), np.asarray(inp["b_post2"], np.float64)
    W_lin, b_lin = np.asarray(inp["W_lin"], np.float64), np.asarray(inp["b_lin"], np.float64)
    W_bond, b_bond = np.asarray(inp["W_bond"], np.float64), np.asarray(inp["b_bond"], np.float64)
    W_ee, b_ee = np.asarray(inp["W_ee"], np.float64), np.asarray(inp["b_ee"], np.float64)
    assert np.allclose(np.asarray(inp["ln_w"]), 1.0) and \
        np.allclose(np.asarray(inp["ln_b"]), 0.0), "ln affine not identity"

    def blockdiag(mats):
        n = len(mats)
        r, co = mats[0].shape
        out = np.zeros((n * r, n * co), dtype=np.float64)
        for t in range(n):
            out[t * r:(t + 1) * r, t * co:(t + 1) * co] = mats[t]
        return out

    W_be = W_bond @ W_ee
    b_be = b_bond @ W_ee + b_ee
    Wxi = blockdiag([W_pre1[t][0:F_IN] for t in range(T)])
    Wxj = blockdiag([W_pre1[t][F_IN:2 * F_IN] for t in range(T)])
    Wea_cat = np.concatenate([W_pre1[t][2 * F_IN:3 * F_IN] for t in range(T)], axis=1)
    W_bea = W_be @ Wea_cat
    b1p = b_pre1.reshape(H) + b_be @ Wea_cat
    W2bd = blockdiag([W_pre2[t] for t in range(T)])
    b2 = b_pre2.reshape(H)
    W1X = blockdiag([W_post1[t][0:F_IN] for t in range(T)])
    W1 = [[blockdiag([W_post1[t][F_IN + r * 5 * F_IN + a * F_IN:
                                 F_IN + r * 5 * F_IN + (a + 1) * F_IN]
                      for t in range(T)]) for a in range(5)] for r in range(3)]
    W2p = blockdiag([W_post2[t] for t in range(T)])
    b3 = b_post2.reshape(H)
    bp1 = b_post1.reshape(H)

    # LN centering fold
    P_c = np.eye(H) - np.ones((H, H)) / H
    WLINc = W_lin @ P_c
    blinc = P_c @ b_lin

    # per-degree-class combined weights + corrections
    wblocks = [Wxi, Wxj, W2bd, np.eye(H), W1X, W2p, WLINc]
    cvecs = []

    def wcomb(d):
        dc = max(d, 1.0)
        logdeg = np.log(dc + 1.0)
        amp, att = logdeg / AVG_DEG_LOG, AVG_DEG_LOG / logdeg
        return [W1[0][a] + amp * W1[1][a] + att * W1[2][a] for a in range(5)]

    # class 0 = degree 0: std = sqrt(eps), everything else zero
    Wc0 = wcomb(0)
    cvecs.append(np.sqrt(EPS) * Wc0[4].sum(axis=0))
    for d in p.dclasses:
        Wc = wcomb(d)
        if d == 1:
            wblocks.append(Wc[0] + Wc[1] + Wc[2] + Wc[3])
            cvecs.append(b2 @ (1 * Wc[0] + Wc[1] + Wc[2] + Wc[3])
                         + np.sqrt(EPS) * Wc[4].sum(axis=0))
        else:
            wblocks.extend([Wc[0] + Wc[1] / d, Wc[2], Wc[3], Wc[4]])
            cvecs.append(b2 @ (d * Wc[0] + Wc[1] + Wc[2] + Wc[3]))

    assert len(wblocks) == p.n_wblocks
    wmm = np.concatenate([np.asarray(w) for w in wblocks], axis=1).astype(BF)
    cvrow = np.concatenate([np.ones(H)] + cvecs).astype(BF)[None, :]
    assert cvrow.shape[1] == p.n_cvrow * 128, (cvrow.shape, p.n_cvrow)
    baux = np.stack([b1p, bp1, b3, blinc], axis=1).astype(np.float32)  # [128,4]
    return dict(
        wmm=np.ascontiguousarray(wmm),
        wbea16=np.ascontiguousarray(W_bea.astype(BF)),
        cvrow=np.ascontiguousarray(cvrow),
        baux=np.ascontiguousarray(baux),
    )


# --------------------------------------------------------------------------
# Bass kernel builder
# --------------------------------------------------------------------------

def build_nc(p, n_nodes, debug=False):
    nc = bacc.Bacc("TRN2", target_bir_lowering=False, debug=debug)
    S, NL = p.S, p.N_layout
    NB = NL // 512

    xjT_d = nc.dram_tensor("xjT", [128, S], BF16, kind="ExternalInput")
    bondT_d = nc.dram_tensor("bondT", [EC, S], BF16, kind="ExternalInput")
    xlt_d = nc.dram_tensor("x_layT", [128, NL], BF16, kind="ExternalInput")
    xln_d = nc.dram_tensor("x_lay", [128, NL], F32, kind="ExternalInput")
    wmm_d = nc.dram_tensor("wmm", [128, p.n_wblocks * 128], BF16, kind="ExternalInput")
    wbea_d = nc.dram_tensor("wbea16", [EC, 128], BF16, kind="ExternalInput")
    cvrow_d = nc.dram_tensor("cvrow", [1, p.n_cvrow * 128], BF16, kind="ExternalInput")
    baux_d = nc.dram_tensor("baux", [128, 4], F32, kind="ExternalInput")
    out_d = nc.dram_tensor("out", [NL, H], F32, kind="ExternalOutput")

    # blocks -> pieces; piece = (d, a, b) with [a,b) within block
    sec_all = [(0, 0, p.n0_max)] + p.sections
    block_pieces = [[] for _ in range(NB)]
    for (d, a, b) in sec_all:
        if b <= a:
            continue
        t0, t1 = a // 512, (b - 1) // 512
        for t in range(t0, t1 + 1):
            pa, pb_ = max(a, t * 512), min(b, (t + 1) * 512)
            if pb_ > pa:
                block_pieces[t].append((d, pa, pb_))

    # block -> last chunk index contributing to it
    last_chunk = [-1] * NB
    for ci, (d, g, cols, sb, nb) in enumerate(p.chunks):
        for t in range(nb // 512, min((nb + g - 1) // 512, NB - 1) + 1):
            last_chunk[t] = max(last_chunk[t], ci)
    # ensure monotone (a block can't be ready before an earlier block's chunks)
    for t in range(1, NB):
        last_chunk[t] = max(last_chunk[t], last_chunk[t - 1])

    with tile.TileContext(nc) as tc:
        from contextlib import ExitStack
        with ExitStack() as ctx:
            cpool = ctx.enter_context(tc.tile_pool(name="consts", bufs=1))
            wmm = cpool.tile([128, p.n_wblocks * 128], BF16)
            nc.sync.dma_start(wmm[:], wmm_d[:])
            wbea16 = cpool.tile([EC, 128], BF16)
            nc.sync.dma_start(wbea16[:], wbea_d[:])
            cvrow = cpool.tile([1, p.n_cvrow * 128], BF16)
            nc.sync.dma_start(cvrow[:], cvrow_d[:])
            baux = cpool.tile([128, 4], F32)
            nc.sync.dma_start(baux[:], baux_d[:])
            ident = cpool.tile([128, 128], F32)
            make_identity(nc, ident[:])
            epsc = cpool.tile([128, 1], F32)
            nc.vector.memset(epsc[:], EPS)
            onescol = cpool.tile([128, 1], BF16)
            nc.vector.memset(onescol[:], 1.0)
            ones512 = cpool.tile([1, 512], BF16)
            nc.vector.memset(ones512[:], 1.0)

            def WB(i):
                return wmm[:, i * 128:(i + 1) * 128]

            def CV(di):
                i = p.cv_idx[di] + 1  # +1: row block 0 is the ones row
                return cvrow[:, i * 128:(i + 1) * 128]

            ONESROW = cvrow[:, 0:128]
            B1P, BP1, B3, BLINC = (baux[:, i:i + 1] for i in range(4))

            # persistent aggregate arrays
            aggp = ctx.enter_context(tc.tile_pool(name="agg", bufs=1))
            s16_agg = aggp.tile([128, NL], BF16)
            mn16_agg = aggp.tile([128, NL], BF16)
            mx16_agg = aggp.tile([128, NL], BF16)
            std16_agg = aggp.tile([128, NL], BF16)

            ep = ctx.enter_context(tc.tile_pool(name="edge_sb", bufs=3))
            sp2 = ctx.enter_context(tc.tile_pool(name="stdchain_sb", bufs=2))
            gp = ctx.enter_context(tc.tile_pool(name="edge_gath", bufs=3))
            pb = ctx.enter_context(tc.tile_pool(name="post_sb", bufs=2))
            epp = ctx.enter_context(tc.tile_pool(name="edge_ps", bufs=4, space="PSUM"))
            sqp = ctx.enter_context(tc.tile_pool(name="sq_ps", bufs=2, space="PSUM"))
            opp = ctx.enter_context(tc.tile_pool(name="op_ps", bufs=1, space="PSUM"))
            mpp = ctx.enter_context(tc.tile_pool(name="misc_ps", bufs=1, space="PSUM"))

            gtiles = {}

            def emit_gather(gi):
                gsb, gcols, _ = p.groups[gi]
                xj_fmg = gp.tile([128, gcols], BF16, tag="xj_fmg")
                gtiles[gi] = xj_fmg
                nc.sync.dma_start(xj_fmg[:], xjT_d[:, gsb:gsb + gcols])

            def emit_head(gi, ci):
                """Independent front of a chunk: p1 matmuls + h1 evac.

                Emitted one chunk ahead of the dependent tail so the in-order
                PE queue always has ready work in front of tail matmuls that
                wait on the scalar h1 evacuation."""
                gsb, gcols, _ = p.groups[gi]
                xj_fmg = gtiles[gi]
                d, g, cols, sb, nb = p.chunks[ci]
                loc = sb - gsb
                gd = g * d
                nsl = slice(nb, nb + g)

                bond_t = ep.tile([EC, 512], BF16, tag="bond")
                nc.sync.dma_start(bond_t[:, 0:gd], bondT_d[:, sb:sb + gd])
                xi_t = ep.tile([128, 512], BF16, tag="xi_t")
                nc.sync.dma_start(xi_t[:, 0:g], xlt_d[:, nsl])

                p1 = epp.tile([128, 512], F32, tag="mm")
                nc.tensor.matmul(out=p1[:, 0:gd], lhsT=wbea16[:],
                                 rhs=bond_t[:, 0:gd], start=True, stop=False)
                nc.tensor.matmul(out=p1[:, 0:gd], lhsT=WB(p.widx["WXJ"]),
                                 rhs=xj_fmg[:, loc:loc + gd], start=False, stop=False)
                nc.tensor.matmul(
                    out=p1[:, 0:gd].rearrange("p (k j) -> p k j", k=d),
                    lhsT=WB(p.widx["WXI"]),
                    rhs=xi_t[:, None, 0:g].to_broadcast((128, d, g)),
                    start=False, stop=True)
                h1 = ep.tile([128, 512], BF16, tag="h1")
                nc.scalar.activation(h1[:, 0:gd], p1[:, 0:gd], AF.Relu, bias=B1P)
                return h1

            def emit_tail(ci, h1):
                d, g, cols, sb, nb = p.chunks[ci]
                gd = g * d
                nsl = slice(nb, nb + g)
                p2 = epp.tile([128, 512], F32, tag="mm")
                nc.tensor.matmul(out=p2[:, 0:gd], lhsT=WB(p.widx["W2BD"]),
                                 rhs=h1[:, 0:gd], start=True, stop=True)

                if d == 1:
                    nc.scalar.activation(s16_agg[:, nsl], p2[:, 0:g], AF.Copy)
                    return

                msq = ep.tile([128, 512], BF16, tag="msq")
                nc.scalar.activation(msq[:, 0:gd], p2[:, 0:gd], AF.Square)

                sq = sqp.tile([128, 512], F32, tag="sq")
                nc.tensor.matmul(
                    out=sq[:, None, 0:g].to_broadcast((128, d, g)),
                    lhsT=WB(p.widx["W2BD"]),
                    rhs=h1[:, 0:gd].rearrange("p (k j) -> p k j", k=d),
                    start=True, stop=True, skip_group_check=True)
                nc.tensor.matmul(
                    out=sq[:, None, 256:256 + g].to_broadcast((128, d, g)),
                    lhsT=WB(p.widx["IDENT"]),
                    rhs=msq[:, 0:gd].rearrange("p (k j) -> p k j", k=d),
                    start=True, stop=True, skip_group_check=True)

                nc.scalar.activation(s16_agg[:, nsl], sq[:, 0:g], AF.Copy)
                msqt_t = sp2.tile([128, 256], F32, tag="msqt")
                nc.scalar.activation(msqt_t[:, 0:g], sq[:, 0:g], AF.Square,
                                     scale=1.0 / d)
                e2t = sp2.tile([128, 256], F32, tag="e2t")
                nc.scalar.activation(e2t[:, 0:g], sq[:, 256:256 + g], AF.Copy,
                                     scale=1.0 / d)
                vt = sp2.tile([128, 256], F32, tag="vt")
                nc.gpsimd.tensor_tensor(vt[:, 0:g], e2t[:, 0:g], msqt_t[:, 0:g],
                                        OP.subtract)
                vt2 = sp2.tile([128, 256], F32, tag="vt2")
                # (var max 0) + eps in one DVE op; then a plain sqrt
                nc.vector.tensor_scalar(vt2[:, 0:g], vt[:, 0:g], 0.0, EPS,
                                        OP.max, OP.add)
                nc.scalar.activation(std16_agg[:, nsl], vt2[:, 0:g], AF.Sqrt)

                p2v = p2[:, 0:gd].rearrange("p (k j) -> p j k", k=d)
                nc.vector.tensor_reduce(out=mn16_agg[:, nsl], in_=p2v,
                                        axis=AX.X, op=OP.min)
                nc.vector.tensor_reduce(out=mx16_agg[:, nsl], in_=p2v,
                                        axis=AX.X, op=OP.max)

            def emit_block(t):
                nb = t * 512
                nsl = slice(nb, nb + 512)
                pieces = block_pieces[t]

                x_fm = pb.tile([128, 512], BF16, tag="x_fm")
                nc.sync.dma_start(x_fm[:], xlt_d[:, nsl])
                x_nm = pb.tile([128, 512], F32, tag="x_nm")
                nc.sync.dma_start(x_nm[:], xln_d[:, nsl])

                op = opp.tile([128, 512], F32, tag="op")
                mms = [dict(out=op[:], lhsT=WB(p.widx["W1X"]), rhs=x_fm[:])]
                for (d, a, b) in pieces:
                    rel = slice(a - nb, b - nb)
                    n = b - a
                    if d == 0:
                        mms.append(dict(out=op[:, rel], lhsT=CV(0),
                                        rhs=ones512[:, 0:n]))
                    elif d == 1:
                        wi = p.wd_idx[1]
                        mms.append(dict(out=op[:, rel], lhsT=WB(wi),
                                        rhs=s16_agg[:, a:b]))
                        mms.append(dict(out=op[:, rel], lhsT=CV(1),
                                        rhs=ones512[:, 0:n]))
                    else:
                        wi = p.wd_idx[d]
                        mms.append(dict(out=op[:, rel], lhsT=WB(wi),
                                        rhs=s16_agg[:, a:b]))
                        mms.append(dict(out=op[:, rel], lhsT=WB(wi + 1),
                                        rhs=mn16_agg[:, a:b]))
                        mms.append(dict(out=op[:, rel], lhsT=WB(wi + 2),
                                        rhs=mx16_agg[:, a:b]))
                        mms.append(dict(out=op[:, rel], lhsT=WB(wi + 3),
                                        rhs=std16_agg[:, a:b]))
                        mms.append(dict(out=op[:, rel], lhsT=CV(d),
                                        rhs=ones512[:, 0:n]))
                for i, kw in enumerate(mms):
                    nc.tensor.matmul(start=(i == 0), stop=(i == len(mms) - 1),
                                     skip_group_check=True, **kw)

                h1p = pb.tile([128, 512], BF16, tag="h1p")
                nc.vector.tensor_scalar(h1p[:], op[:], BP1, 0.0, OP.add, OP.max)
                pp2 = mpp.tile([128, 512], F32, tag="mp")
                nc.tensor.matmul(out=pp2[:], lhsT=WB(p.widx["W2P"]), rhs=h1p[:],
                                 start=True, stop=True)
                z2 = pb.tile([128, 512], BF16, tag="z2")
                nc.vector.tensor_scalar(z2[:], pp2[:], B3, None, OP.add)
                plin = mpp.tile([128, 512], F32, tag="mp")
                nc.tensor.matmul(out=plin[:], lhsT=WB(p.widx["WLINC"]), rhs=z2[:],
                                 start=True, stop=True)
                zf = pb.tile([128, 512], F32, tag="zf")
                nc.scalar.activation(zf[:], plin[:], AF.Identity, bias=BLINC)
                sq16 = pb.tile([128, 512], BF16, tag="sq16")
                nc.gpsimd.tensor_tensor(sq16[:], zf[:], zf[:], OP.mult)
                vs = mpp.tile([128, 512], F32, tag="mp")
                nc.tensor.matmul(out=vs[0:1, :], lhsT=onescol[:], rhs=sq16[:],
                                 start=True, stop=True)
                sd = pb.tile([1, 512], F32, tag="sd")
                nc.scalar.activation(sd[:], vs[0:1, :], AF.Sqrt,
                                     scale=1.0 / 128.0, bias=epsc[0:1, :])
                ri32 = pb.tile([1, 512], F32, tag="ri32")
                nc.vector.reciprocal_approx_fast(ri32[:], sd[:])
                ri = pb.tile([1, 512], BF16, tag="ri")
                with nc.allow_low_precision(reason="rstd broadcast via bf16 ones matmul"):
                    nc.vector.tensor_copy(ri[:], ri32[:])
                rb = mpp.tile([128, 512], F32, tag="mp")
                nc.tensor.matmul(out=rb[:], lhsT=ONESROW, rhs=ri[:],
                                 start=True, stop=True)
                y = pb.tile([128, 512], F32, tag="y")
                nc.vector.tensor_tensor(y[:], zf[:], rb[:], OP.mult)
                ry = pb.tile([128, 512], F32, tag="ry")
                nc.vector.tensor_scalar(ry[:], y[:], 0.0, None, OP.max)
                zps = mpp.tile([128, 512], F32, tag="mp")
                for bq in range(4):
                    sl = slice(128 * bq, 128 * (bq + 1))
                    nc.tensor.transpose(out=zps[:, sl], in_=ry[:, sl],
                                        identity=ident[:])
                out_nm = pb.tile([128, 512], F32, tag="out_nm")
                nc.vector.tensor_tensor(out_nm[:], zps[:], x_nm[:], OP.add)
                nc.sync.dma_start(
                    out_d[nsl, :].rearrange("(j p) f -> p j f", p=128),
                    out_nm[:].rearrange("p (j f) -> p j f", f=128))

            # interleaved, software-pipelined emission: head(i+1) before tail(i)
            next_block = 0

            def emit_tail_and_blocks(item):
                nonlocal next_block
                ci, h1 = item
                emit_tail(ci, h1)
                while next_block < NB and last_chunk[next_block] <= ci:
                    emit_block(next_block)
                    next_block += 1

            pending = None  # (ci, h1) awaiting tail
            for gi in range(len(p.groups)):
                emit_gather(gi)
                for ci in p.groups[gi][2]:
                    h1 = emit_head(gi, ci)
                    if pending is not None:
                        emit_tail_and_blocks(pending)
                    pending = (ci, h1)
            if pending is not None:
                emit_tail_and_blocks(pending)
            while next_block < NB:
                emit_block(next_block)
                next_block += 1
    nc.compile()
    return nc


# --------------------------------------------------------------------------
# Entry point
# --------------------------------------------------------------------------

_CACHE = {}


def _get_compiled(src, dst, n_nodes):
    key = hash((src.tobytes(), dst.tobytes(), n_nodes))
    if key not in _CACHE:
        p = make_plan(src.astype(np.int64), dst.astype(np.int64), n_nodes)
        nc = build_nc(p, n_nodes)
        _CACHE[key] = (p, nc)
    return _CACHE[key]


def kernel(**inputs):
    atom_x = np.asarray(inputs["atom_x"], np.float32)
    bond_x = np.asarray(inputs["bond_x"], np.float32)
    ei = np.asarray(inputs["atom_edge_index"])
    src = ei[0].astype(np.int64)
    n_nodes = atom_x.shape[0]

    p, nc = _get_compiled(ei[0], ei[1], n_nodes)
    W = make_weights(inputs, p)
    in_maps = [make_core_inputs(p, c, atom_x, bond_x, src, W)
               for c in range(p.n_cores)]
    res = run_bass_kernel_spmd(nc, in_maps, core_ids=list(range(p.n_cores)))

    out = np.zeros((n_nodes, H), dtype=np.float32)
    for c in range(p.n_cores):
        o = res.results[c]["out"]
        lay = p.layout_nodes[c]
        real = np.nonzero(lay >= 0)[0]
        out[c * p.npc + lay[real]] = o[real]
    return out


# revision 33
# speedup vs baseline: 1.5455x; 1.0304x over previous
"""Trainium2 Bass kernel for nn_Drug_PNAConv (GNN message passing, PNAConv).

v3c strategy:
  - Nodes partitioned by destination across 8 cores; host bins edges by dest
    degree into fixed chunks (g nodes x degree d), slots k-major ([d, g]).
  - Source features pre-gathered ON HOST into a contiguous feature-major bf16
    stream xjT [128, S]; no device gather at all.
  - Edge phase per chunk: p1 = Wbea.bond + Wxj.xj + Wxi.xi (PSUM), h1 = relu
    (scalar evac), p2 = W2BD.h1.  Segmented SUM and SUMSQ via single matmuls
    with stride-0 output APs (PSUM self-accumulation); min/max via DVE
    tensor_reduce directly from p2 PSUM (raw, pre-bias).
  - b_pre2 never applied per-edge: sum/mean/min/max offsets are folded into
    per-degree-class correction vectors (rank-1 ones matmuls); variance is
    shift-invariant so the std chain uses raw moments.
  - Degree scalers (identity/amplification/attenuation) folded into
    per-degree-class combined post weights W(d,a) = W1(0,a) + amp_d*W1(1,a)
    + att_d*W1(2,a); mean folded into the s-weight (Wsm = Wc0 + Wc1/d).
    d=0 and d=1 collapse further (std=sqrt(eps) exactly).
  - LayerNorm mean-centering folded into W_lin on host (P_c = I - 11^T/128);
    variance via ones-column matmul; rstd broadcast via ones-row matmul.
  - Edge and post phases interleaved per 512-node block for engine overlap.
"""

import os
import sys

for _p in ("/opt/trn_rl_repo", os.path.expanduser("~/.axon_site/_ro/trn_rl_repo")):
    if os.path.isdir(_p) and _p not in sys.path:
        sys.path.insert(0, _p)

import numpy as np

import concourse.bass as bass
import concourse.bacc as bacc
import concourse.mybir as mybir
import concourse.tile as tile
from concourse.bass_utils import run_bass_kernel_spmd
from concourse.masks import make_identity

F32 = mybir.dt.float32
BF16 = mybir.dt.bfloat16
AF = mybir.ActivationFunctionType
OP = mybir.AluOpType
AX = mybir.AxisListType

N_CORES = 8
H = 128
T = 4
F_IN = 32
EC = 16
EPS = 1e-5
GROUP_COLS = 2048

_DEG_HIST = np.array([0.0, 5000.0, 20000.0, 25000.0, 10000.0])
_BINS = np.arange(_DEG_HIST.size)
AVG_DEG_LOG = float((np.log(_BINS + 1.0) * _DEG_HIST).sum() / _DEG_HIST.sum())


def _ceil_to(x, m):
    return ((x + m - 1) // m) * m


# --------------------------------------------------------------------------
# Host-side planning (sharding + layout)
# --------------------------------------------------------------------------

class Plan:
    pass


def make_plan(src, dst, n_nodes, n_cores=N_CORES):
    assert n_nodes % n_cores == 0
    npc = n_nodes // n_cores
    p = Plan()
    p.n_nodes = n_nodes
    p.npc = npc
    p.n_cores = n_cores

    owner = dst // npc
    core_edges = []
    core_deg = []
    dmax = 0
    for c in range(n_cores):
        sel = np.nonzero(owner == c)[0]
        dloc = dst[sel] - c * npc
        deg = np.bincount(dloc, minlength=npc)
        dmax = max(dmax, int(deg.max()) if deg.size else 0)
        core_edges.append(sel)
        core_deg.append(deg)
    assert dmax <= 512, f"degree {dmax} too large"
    p.dmax = dmax

    n_d_max = np.zeros(dmax + 1, dtype=np.int64)
    for c in range(n_cores):
        cnt = np.bincount(core_deg[c], minlength=dmax + 1)
        n_d_max = np.maximum(n_d_max, cnt)

    sec_size = [int(n_d_max[0])] + [int(n_d_max[d]) for d in range(1, dmax + 1)]
    sec_off = np.concatenate([[0], np.cumsum(sec_size)])
    n_used = int(sec_off[-1])
    p.N_layout = _ceil_to(max(n_used, 512), 512)
    p.n0_max = int(n_d_max[0])
    p.n_used = n_used

    chunks = []  # (d, g, cols, slot_base, node_base)
    sbase = 0
    for d in range(1, dmax + 1):
        rem = int(n_d_max[d])
        nbase = int(sec_off[d])
        gmax = 512 // d
        while rem > 0:
            g = min(rem, gmax)
            cols = _ceil_to(g * d, 128)
            chunks.append((d, g, cols, sbase, nbase))
            sbase += cols
            nbase += g
            rem -= g
    p.chunks = chunks
    p.S = sbase if sbase > 0 else 128

    # sections: per degree-class node range (d >= 1, nonempty)
    p.sections = []
    for d in range(1, dmax + 1):
        if n_d_max[d] > 0:
            p.sections.append((d, int(sec_off[d]), int(sec_off[d + 1])))
    p.dclasses = [d for (d, a, b) in p.sections]

    # gather groups of consecutive chunks, total cols <= GROUP_COLS
    groups = []
    cur = None
    for ci, (d, g, cols, sb, nb) in enumerate(chunks):
        if cur is None or cur[1] + cols > GROUP_COLS:
            cur = [sb, cols, [ci]]
            groups.append(cur)
        else:
            cur[1] += cols
            cur[2].append(ci)
    p.groups = [tuple(x) for x in groups]

    # per-core node layout + slot->edge map (k-major within chunks)
    p.layout_nodes = []
    p.core_edges_sorted = []
    for c in range(n_cores):
        deg = core_deg[c]
        lay = np.full(p.N_layout, -1, dtype=np.int64)
        for d in range(0, dmax + 1):
            ids = np.nonzero(deg == d)[0]
            lay[sec_off[d]:sec_off[d] + ids.size] = ids
        p.layout_nodes.append(lay)

        sel = core_edges[c]
        dloc = dst[sel] - c * npc
        eorder = np.argsort(dloc, kind="stable")
        sel_sorted = sel[eorder]
        starts = np.zeros(npc + 1, dtype=np.int64)
        starts[1:] = np.cumsum(deg)

        slot_edge = np.full(p.S, -1, dtype=np.int64)
        for (d, g, cols, sb, nb) in chunks:
            nodes = lay[nb:nb + g]
            real = np.nonzero(nodes >= 0)[0]
            ed = np.full((g, d), -1, dtype=np.int64)
            if real.size:
                rn = nodes[real]
                em = starts[rn][:, None] + np.arange(d)[None, :]
                ed[real] = sel_sorted[em]
            # k-major: slot = k*g + j
            slot_edge[sb:sb + g * d] = ed.T.ravel()
        p.core_edges_sorted.append(slot_edge)

    # weight-blob index layout (structure only; values filled by make_weights)
    p.widx = dict(WXI=0, WXJ=1, W2BD=2, IDENT=3, W1X=4, W2P=5, WLINC=6)
    p.wd_idx = {}
    p.cv_idx = {0: 0}
    bi, ci = 7, 1
    for d in p.dclasses:
        p.wd_idx[d] = bi
        bi += 1 if d == 1 else 4
        p.cv_idx[d] = ci
        ci += 1
    p.n_wblocks = bi
    p.n_cvrow = 1 + ci
    return p


def make_core_inputs(p, c, atom_x, bond_x, src, W):
    npc = p.npc
    lay = p.layout_nodes[c]
    slot_edge = p.core_edges_sorted[c]
    S = p.S

    import ml_dtypes
    valid = slot_edge >= 0
    se = np.maximum(slot_edge, 0)
    xj_id = np.where(valid, src[se], 0)
    xj = atom_x[xj_id]
    xj[~valid] = 0.0
    xjT = np.ascontiguousarray(xj.T.astype(ml_dtypes.bfloat16))

    bondT = np.zeros((S, EC), dtype=ml_dtypes.bfloat16)
    bondT[valid] = bond_x[slot_edge[valid]].astype(ml_dtypes.bfloat16)
    bondT = np.ascontiguousarray(bondT.T)

    gid = np.where(lay >= 0, c * npc + lay, 0)
    xl = atom_x[gid]                                    # [NL, 128]
    x_layT = np.ascontiguousarray(xl.T.astype(ml_dtypes.bfloat16))
    x_lay = np.ascontiguousarray(
        xl.reshape(p.N_layout // 128, 128, H).transpose(1, 0, 2).reshape(128, -1))

    m = dict(xjT=xjT, bondT=bondT, x_layT=x_layT, x_lay=x_lay)
    m.update(W)
    return m


def make_weights(inp, p):
    """Host-side weight folding. Returns dict of shared DRAM inputs."""
    import ml_dtypes
    BF = ml_dtypes.bfloat16
    W_pre1, b_pre1 = np.asarray(inp["W_pre1"], np.float64), np.asarray(inp["b_pre1"], np.float64)
    W_pre2, b_pre2 = np.asarray(inp["W_pre2"], np.float64), np.asarray(inp["b_pre2"], np.float64)
    W_post1, b_post1 = np.asarray(inp["W_post1"], np.float64), np.asarray(inp["b_post1"], np.float64)
    W_post2, b_post2 = np.asarray(inp["W_post2"], np.float64), np.asarray(inp["b_post2"], np.float64)
    W_lin, b_lin = np.asarray(inp["W_lin"], np.float64), np.asarray(inp["b_lin"], np.float64)
    W_bond, b_bond = np.asarray(inp["W_bond"], np.float64), np.asarray(inp["b_bond"], np.float64)
    W_ee, b_ee = np.asarray(inp["W_ee"], np.float64), np.asarray(inp["b_ee"], np.float64)
    assert np.allclose(np.asarray(inp["ln_w"]), 1.0) and \
        np.allclose(np.asarray(inp["ln_b"]), 0.0), "ln affine not identity"

    def blockdiag(mats):
        n = len(mats)
        r, co = mats[0].shape
        out = np.zeros((n * r, n * co), dtype=np.float64)
        for t in range(n):
            out[t * r:(t + 1) * r, t * co:(t + 1) * co] = mats[t]
        return out

    W_be = W_bond @ W_ee
    b_be = b_bond @ W_ee + b_ee
    Wxi = blockdiag([W_pre1[t][0:F_IN] for t in range(T)])
    Wxj = blockdiag([W_pre1[t][F_IN:2 * F_IN] for t in range(T)])
    Wea_cat = np.concatenate([W_pre1[t][2 * F_IN:3 * F_IN] for t in range(T)], axis=1)
    W_bea = W_be @ Wea_cat
    b1p = b_pre1.reshape(H) + b_be @ Wea_cat
    W2bd = blockdiag([W_pre2[t] for t in range(T)])
    b2 = b_pre2.reshape(H)
    W1X = blockdiag([W_post1[t][0:F_IN] for t in range(T)])
    W1 = [[blockdiag([W_post1[t][F_IN + r * 5 * F_IN + a * F_IN:
                                 F_IN + r * 5 * F_IN + (a + 1) * F_IN]
                      for t in range(T)]) for a in range(5)] for r in range(3)]
    W2p = blockdiag([W_post2[t] for t in range(T)])
    b3 = b_post2.reshape(H)
    bp1 = b_post1.reshape(H)

    # LN centering fold
    P_c = np.eye(H) - np.ones((H, H)) / H
    WLINc = W_lin @ P_c
    blinc = P_c @ b_lin

    # per-degree-class combined weights + corrections
    wblocks = [Wxi, Wxj, W2bd, np.eye(H), W1X, W2p, WLINc]
    cvecs = []

    def wcomb(d):
        dc = max(d, 1.0)
        logdeg = np.log(dc + 1.0)
        amp, att = logdeg / AVG_DEG_LOG, AVG_DEG_LOG / logdeg
        return [W1[0][a] + amp * W1[1][a] + att * W1[2][a] for a in range(5)]

    # class 0 = degree 0: std = sqrt(eps), everything else zero
    Wc0 = wcomb(0)
    cvecs.append(np.sqrt(EPS) * Wc0[4].sum(axis=0))
    for d in p.dclasses:
        Wc = wcomb(d)
        if d == 1:
            wblocks.append(Wc[0] + Wc[1] + Wc[2] + Wc[3])
            cvecs.append(b2 @ (1 * Wc[0] + Wc[1] + Wc[2] + Wc[3])
                         + np.sqrt(EPS) * Wc[4].sum(axis=0))
        else:
            wblocks.extend([Wc[0] + Wc[1] / d, Wc[2], Wc[3], Wc[4]])
            cvecs.append(b2 @ (d * Wc[0] + Wc[1] + Wc[2] + Wc[3]))

    assert len(wblocks) == p.n_wblocks
    wmm = np.concatenate([np.asarray(w) for w in wblocks], axis=1).astype(BF)
    cvrow = np.concatenate([np.ones(H)] + cvecs).astype(BF)[None, :]
    assert cvrow.shape[1] == p.n_cvrow * 128, (cvrow.shape, p.n_cvrow)
    baux = np.stack([b1p, bp1, b3, blinc], axis=1).astype(np.float32)  # [128,4]
    return dict(
        wmm=np.ascontiguousarray(wmm),
        wbea16=np.ascontiguousarray(W_bea.astype(BF)),
        cvrow=np.ascontiguousarray(cvrow),
        baux=np.ascontiguousarray(baux),
    )


# --------------------------------------------------------------------------
# Bass kernel builder
# --------------------------------------------------------------------------

def build_nc(p, n_nodes, debug=False):
    nc = bacc.Bacc("TRN2", target_bir_lowering=False, debug=debug)
    S, NL = p.S, p.N_layout
    NB = NL // 512

    xjT_d = nc.dram_tensor("xjT", [128, S], BF16, kind="ExternalInput")
    bondT_d = nc.dram_tensor("bondT", [EC, S], BF16, kind="ExternalInput")
    xlt_d = nc.dram_tensor("x_layT", [128, NL], BF16, kind="ExternalInput")
    xln_d = nc.dram_tensor("x_lay", [128, NL], F32, kind="ExternalInput")
    wmm_d = nc.dram_tensor("wmm", [128, p.n_wblocks * 128], BF16, kind="ExternalInput")
    wbea_d = nc.dram_tensor("wbea16", [EC, 128], BF16, kind="ExternalInput")
    cvrow_d = nc.dram_tensor("cvrow", [1, p.n_cvrow * 128], BF16, kind="ExternalInput")
    baux_d = nc.dram_tensor("baux", [128, 4], F32, kind="ExternalInput")
    out_d = nc.dram_tensor("out", [NL, H], F32, kind="ExternalOutput")

    # blocks -> pieces; piece = (d, a, b) with [a,b) within block
    sec_all = [(0, 0, p.n0_max)] + p.sections
    block_pieces = [[] for _ in range(NB)]
    for (d, a, b) in sec_all:
        if b <= a:
            continue
        t0, t1 = a // 512, (b - 1) // 512
        for t in range(t0, t1 + 1):
            pa, pb_ = max(a, t * 512), min(b, (t + 1) * 512)
            if pb_ > pa:
                block_pieces[t].append((d, pa, pb_))

    # block -> last chunk index contributing to it
    last_chunk = [-1] * NB
    for ci, (d, g, cols, sb, nb) in enumerate(p.chunks):
        for t in range(nb // 512, min((nb + g - 1) // 512, NB - 1) + 1):
            last_chunk[t] = max(last_chunk[t], ci)
    # ensure monotone (a block can't be ready before an earlier block's chunks)
    for t in range(1, NB):
        last_chunk[t] = max(last_chunk[t], last_chunk[t - 1])

    with tile.TileContext(nc) as tc:
        from contextlib import ExitStack
        with ExitStack() as ctx:
            cpool = ctx.enter_context(tc.tile_pool(name="consts", bufs=1))
            wmm = cpool.tile([128, p.n_wblocks * 128], BF16)
            nc.sync.dma_start(wmm[:], wmm_d[:])
            wbea16 = cpool.tile([EC, 128], BF16)
            nc.sync.dma_start(wbea16[:], wbea_d[:])
            cvrow = cpool.tile([1, p.n_cvrow * 128], BF16)
            nc.sync.dma_start(cvrow[:], cvrow_d[:])
            baux = cpool.tile([128, 4], F32)
            nc.sync.dma_start(baux[:], baux_d[:])
            ident = cpool.tile([128, 128], F32)
            make_identity(nc, ident[:])
            epsc = cpool.tile([128, 1], F32)
            nc.vector.memset(epsc[:], EPS)
            onescol = cpool.tile([128, 1], BF16)
            nc.vector.memset(onescol[:], 1.0)
            ones512 = cpool.tile([1, 512], BF16)
            nc.vector.memset(ones512[:], 1.0)

            def WB(i):
                return wmm[:, i * 128:(i + 1) * 128]

            def CV(di):
                i = p.cv_idx[di] + 1  # +1: row block 0 is the ones row
                return cvrow[:, i * 128:(i + 1) * 128]

            ONESROW = cvrow[:, 0:128]
            B1P, BP1, B3, BLINC = (baux[:, i:i + 1] for i in range(4))

            # persistent aggregate arrays
            aggp = ctx.enter_context(tc.tile_pool(name="agg", bufs=1))
            s16_agg = aggp.tile([128, NL], BF16)
            mn16_agg = aggp.tile([128, NL], BF16)
            mx16_agg = aggp.tile([128, NL], BF16)
            std16_agg = aggp.tile([128, NL], BF16)

            ep = ctx.enter_context(tc.tile_pool(name="edge_sb", bufs=4))
            sp2 = ctx.enter_context(tc.tile_pool(name="stdchain_sb", bufs=2))
            gp = ctx.enter_context(tc.tile_pool(name="edge_gath", bufs=3))
            pb = ctx.enter_context(tc.tile_pool(name="post_sb", bufs=2))
            epp = ctx.enter_context(tc.tile_pool(name="edge_ps", bufs=4, space="PSUM"))
            sqp = ctx.enter_context(tc.tile_pool(name="sq_ps", bufs=2, space="PSUM"))
            opp = ctx.enter_context(tc.tile_pool(name="op_ps", bufs=1, space="PSUM"))
            mpp = ctx.enter_context(tc.tile_pool(name="misc_ps", bufs=1, space="PSUM"))

            gtiles = {}

            def emit_gather(gi):
                gsb, gcols, _ = p.groups[gi]
                xj_fmg = gp.tile([128, gcols], BF16, tag="xj_fmg")
                gtiles[gi] = xj_fmg
                nc.sync.dma_start(xj_fmg[:], xjT_d[:, gsb:gsb + gcols])

            def emit_head(gi, ci):
                """Independent front of a chunk: p1 matmuls + h1 evac.

                Emitted one chunk ahead of the dependent tail so the in-order
                PE queue always has ready work in front of tail matmuls that
                wait on the scalar h1 evacuation."""
                gsb, gcols, _ = p.groups[gi]
                xj_fmg = gtiles[gi]
                d, g, cols, sb, nb = p.chunks[ci]
                loc = sb - gsb
                gd = g * d
                nsl = slice(nb, nb + g)

                bond_t = ep.tile([EC, 512], BF16, tag="bond")
                nc.sync.dma_start(bond_t[:, 0:gd], bondT_d[:, sb:sb + gd])
                xi_t = ep.tile([128, 512], BF16, tag="xi_t")
                nc.sync.dma_start(xi_t[:, 0:g], xlt_d[:, nsl])

                p1 = epp.tile([128, 512], F32, tag="mm")
                nc.tensor.matmul(out=p1[:, 0:gd], lhsT=wbea16[:],
                                 rhs=bond_t[:, 0:gd], start=True, stop=False)
                nc.tensor.matmul(out=p1[:, 0:gd], lhsT=WB(p.widx["WXJ"]),
                                 rhs=xj_fmg[:, loc:loc + gd], start=False, stop=False)
                nc.tensor.matmul(
                    out=p1[:, 0:gd].rearrange("p (k j) -> p k j", k=d),
                    lhsT=WB(p.widx["WXI"]),
                    rhs=xi_t[:, None, 0:g].to_broadcast((128, d, g)),
                    start=False, stop=True)
                h1 = ep.tile([128, 512], BF16, tag="h1")
                nc.scalar.activation(h1[:, 0:gd], p1[:, 0:gd], AF.Relu, bias=B1P)
                return h1

            def emit_tail(ci, h1):
                d, g, cols, sb, nb = p.chunks[ci]
                gd = g * d
                nsl = slice(nb, nb + g)
                p2 = epp.tile([128, 512], F32, tag="mm")
                nc.tensor.matmul(out=p2[:, 0:gd], lhsT=WB(p.widx["W2BD"]),
                                 rhs=h1[:, 0:gd], start=True, stop=True)

                if d == 1:
                    nc.scalar.activation(s16_agg[:, nsl], p2[:, 0:g], AF.Copy)
                    return

                msq = ep.tile([128, 512], BF16, tag="msq")
                nc.scalar.activation(msq[:, 0:gd], p2[:, 0:gd], AF.Square)

                sq = sqp.tile([128, 512], F32, tag="sq")
                nc.tensor.matmul(
                    out=sq[:, None, 0:g].to_broadcast((128, d, g)),
                    lhsT=WB(p.widx["W2BD"]),
                    rhs=h1[:, 0:gd].rearrange("p (k j) -> p k j", k=d),
                    start=True, stop=True, skip_group_check=True)
                nc.tensor.matmul(
                    out=sq[:, None, 256:256 + g].to_broadcast((128, d, g)),
                    lhsT=WB(p.widx["IDENT"]),
                    rhs=msq[:, 0:gd].rearrange("p (k j) -> p k j", k=d),
                    start=True, stop=True, skip_group_check=True)

                nc.scalar.activation(s16_agg[:, nsl], sq[:, 0:g], AF.Copy)
                msqt_t = sp2.tile([128, 256], F32, tag="msqt")
                nc.scalar.activation(msqt_t[:, 0:g], sq[:, 0:g], AF.Square,
                                     scale=1.0 / d)
                e2t = sp2.tile([128, 256], F32, tag="e2t")
                nc.scalar.activation(e2t[:, 0:g], sq[:, 256:256 + g], AF.Copy,
                                     scale=1.0 / d)
                vt = sp2.tile([128, 256], F32, tag="vt")
                nc.gpsimd.tensor_tensor(vt[:, 0:g], e2t[:, 0:g], msqt_t[:, 0:g],
                                        OP.subtract)
                vt2 = sp2.tile([128, 256], F32, tag="vt2")
                # (var max 0) + eps in one DVE op; then a plain sqrt
                nc.vector.tensor_scalar(vt2[:, 0:g], vt[:, 0:g], 0.0, EPS,
                                        OP.max, OP.add)
                nc.scalar.activation(std16_agg[:, nsl], vt2[:, 0:g], AF.Sqrt)

                p2v = p2[:, 0:gd].rearrange("p (k j) -> p j k", k=d)
                nc.vector.tensor_reduce(out=mn16_agg[:, nsl], in_=p2v,
                                        axis=AX.X, op=OP.min)
                nc.vector.tensor_reduce(out=mx16_agg[:, nsl], in_=p2v,
                                        axis=AX.X, op=OP.max)

            def emit_block(t):
                nb = t * 512
                nsl = slice(nb, nb + 512)
                pieces = block_pieces[t]

                x_fm = pb.tile([128, 512], BF16, tag="x_fm")
                nc.sync.dma_start(x_fm[:], xlt_d[:, nsl])
                x_nm = pb.tile([128, 512], F32, tag="x_nm")
                nc.sync.dma_start(x_nm[:], xln_d[:, nsl])

                op = opp.tile([128, 512], F32, tag="op")
                mms = [dict(out=op[:], lhsT=WB(p.widx["W1X"]), rhs=x_fm[:])]
                for (d, a, b) in pieces:
                    rel = slice(a - nb, b - nb)
                    n = b - a
                    if d == 0:
                        mms.append(dict(out=op[:, rel], lhsT=CV(0),
                                        rhs=ones512[:, 0:n]))
                    elif d == 1:
                        wi = p.wd_idx[1]
                        mms.append(dict(out=op[:, rel], lhsT=WB(wi),
                                        rhs=s16_agg[:, a:b]))
                        mms.append(dict(out=op[:, rel], lhsT=CV(1),
                                        rhs=ones512[:, 0:n]))
                    else:
                        wi = p.wd_idx[d]
                        mms.append(dict(out=op[:, rel], lhsT=WB(wi),
                                        rhs=s16_agg[:, a:b]))
                        mms.append(dict(out=op[:, rel], lhsT=WB(wi + 1),
                                        rhs=mn16_agg[:, a:b]))
                        mms.append(dict(out=op[:, rel], lhsT=WB(wi + 2),
                                        rhs=mx16_agg[:, a:b]))
                        mms.append(dict(out=op[:, rel], lhsT=WB(wi + 3),
                                        rhs=std16_agg[:, a:b]))
                        mms.append(dict(out=op[:, rel], lhsT=CV(d),
                                        rhs=ones512[:, 0:n]))
                for i, kw in enumerate(mms):
                    nc.tensor.matmul(start=(i == 0), stop=(i == len(mms) - 1),
                                     skip_group_check=True, **kw)

                h1p = pb.tile([128, 512], BF16, tag="h1p")
                nc.vector.tensor_scalar(h1p[:], op[:], BP1, 0.0, OP.add, OP.max)
                pp2 = mpp.tile([128, 512], F32, tag="mp")
                nc.tensor.matmul(out=pp2[:], lhsT=WB(p.widx["W2P"]), rhs=h1p[:],
                                 start=True, stop=True)
                z2 = pb.tile([128, 512], BF16, tag="z2")
                nc.vector.tensor_scalar(z2[:], pp2[:], B3, None, OP.add)
                plin = mpp.tile([128, 512], F32, tag="mp")
                nc.tensor.matmul(out=plin[:], lhsT=WB(p.widx["WLINC"]), rhs=z2[:],
                                 start=True, stop=True)
                zf = pb.tile([128, 512], F32, tag="zf")
                nc.scalar.activation(zf[:], plin[:], AF.Identity, bias=BLINC)
                sq16 = pb.tile([128, 512], BF16, tag="sq16")
                nc.gpsimd.tensor_tensor(sq16[:], zf[:], zf[:], OP.mult)
                vs = mpp.tile([128, 512], F32, tag="mp")
                nc.tensor.matmul(out=vs[0:1, :], lhsT=onescol[:], rhs=sq16[:],
                                 start=True, stop=True)
                sd = pb.tile([1, 512], F32, tag="sd")
                nc.scalar.activation(sd[:], vs[0:1, :], AF.Sqrt,
                                     scale=1.0 / 128.0, bias=epsc[0:1, :])
                ri32 = pb.tile([1, 512], F32, tag="ri32")
                nc.vector.reciprocal_approx_fast(ri32[:], sd[:])
                ri = pb.tile([1, 512], BF16, tag="ri")
                with nc.allow_low_precision(reason="rstd broadcast via bf16 ones matmul"):
                    nc.vector.tensor_copy(ri[:], ri32[:])
                rb = mpp.tile([128, 512], F32, tag="mp")
                nc.tensor.matmul(out=rb[:], lhsT=ONESROW, rhs=ri[:],
                                 start=True, stop=True)
                y = pb.tile([128, 512], F32, tag="y")
                nc.vector.tensor_tensor(y[:], zf[:], rb[:], OP.mult)
                ry = pb.tile([128, 512], F32, tag="ry")
                nc.vector.tensor_scalar(ry[:], y[:], 0.0, None, OP.max)
                zps = mpp.tile([128, 512], F32, tag="mp")
                for bq in range(4):
                    sl = slice(128 * bq, 128 * (bq + 1))
                    nc.tensor.transpose(out=zps[:, sl], in_=ry[:, sl],
                                        identity=ident[:])
                out_nm = pb.tile([128, 512], F32, tag="out_nm")
                nc.vector.tensor_tensor(out_nm[:], zps[:], x_nm[:], OP.add)
                nc.sync.dma_start(
                    out_d[nsl, :].rearrange("(j p) f -> p j f", p=128),
                    out_nm[:].rearrange("p (j f) -> p j f", f=128))

            # interleaved, software-pipelined emission: head(i+1) before tail(i)
            next_block = 0

            def emit_tail_and_blocks(item):
                nonlocal next_block
                ci, h1 = item
                emit_tail(ci, h1)
                while next_block < NB and last_chunk[next_block] <= ci:
                    emit_block(next_block)
                    next_block += 1

            DEPTH = 2
            pending = []  # [(ci, h1), ...] awaiting tails
            for gi in range(len(p.groups)):
                emit_gather(gi)
                for ci in p.groups[gi][2]:
                    h1 = emit_head(gi, ci)
                    pending.append((ci, h1))
                    if len(pending) > DEPTH:
                        emit_tail_and_blocks(pending.pop(0))
            for item in pending:
                emit_tail_and_blocks(item)
            while next_block < NB:
                emit_block(next_block)
                next_block += 1
    nc.compile()
    return nc


# --------------------------------------------------------------------------
# Entry point
# --------------------------------------------------------------------------

_CACHE = {}


def _get_compiled(src, dst, n_nodes):
    key = hash((src.tobytes(), dst.tobytes(), n_nodes))
    if key not in _CACHE:
        p = make_plan(src.astype(np.int64), dst.astype(np.int64), n_nodes)
        nc = build_nc(p, n_nodes)
        _CACHE[key] = (p, nc)
    return _CACHE[key]


def kernel(**inputs):
    atom_x = np.asarray(inputs["atom_x"], np.float32)
    bond_x = np.asarray(inputs["bond_x"], np.float32)
    ei = np.asarray(inputs["atom_edge_index"])
    src = ei[0].astype(np.int64)
    n_nodes = atom_x.shape[0]

    p, nc = _get_compiled(ei[0], ei[1], n_nodes)
    W = make_weights(inputs, p)
    in_maps = [make_core_inputs(p, c, atom_x, bond_x, src, W)
               for c in range(p.n_cores)]
    res = run_bass_kernel_spmd(nc, in_maps, core_ids=list(range(p.n_cores)))

    out = np.zeros((n_nodes, H), dtype=np.float32)
    for c in range(p.n_cores):
        o = res.results[c]["out"]
        lay = p.layout_nodes[c]
        real = np.nonzero(lay >= 0)[0]
        out[c * p.npc + lay[real]] = o[real]
    return out


# revision 34
# speedup vs baseline: 1.5610x; 1.0100x over previous
"""Trainium2 Bass kernel for nn_Drug_PNAConv (GNN message passing, PNAConv).

v3c strategy:
  - Nodes partitioned by destination across 8 cores; host bins edges by dest
    degree into fixed chunks (g nodes x degree d), slots k-major ([d, g]).
  - Source features pre-gathered ON HOST into a contiguous feature-major bf16
    stream xjT [128, S]; no device gather at all.
  - Edge phase per chunk: p1 = Wbea.bond + Wxj.xj + Wxi.xi (PSUM), h1 = relu
    (scalar evac), p2 = W2BD.h1.  Segmented SUM and SUMSQ via single matmuls
    with stride-0 output APs (PSUM self-accumulation); min/max via DVE
    tensor_reduce directly from p2 PSUM (raw, pre-bias).
  - b_pre2 never applied per-edge: sum/mean/min/max offsets are folded into
    per-degree-class correction vectors (rank-1 ones matmuls); variance is
    shift-invariant so the std chain uses raw moments.
  - Degree scalers (identity/amplification/attenuation) folded into
    per-degree-class combined post weights W(d,a) = W1(0,a) + amp_d*W1(1,a)
    + att_d*W1(2,a); mean folded into the s-weight (Wsm = Wc0 + Wc1/d).
    d=0 and d=1 collapse further (std=sqrt(eps) exactly).
  - LayerNorm mean-centering folded into W_lin on host (P_c = I - 11^T/128);
    variance via ones-column matmul; rstd broadcast via ones-row matmul.
  - Edge and post phases interleaved per 512-node block for engine overlap.
"""

import os
import sys

for _p in ("/opt/trn_rl_repo", os.path.expanduser("~/.axon_site/_ro/trn_rl_repo")):
    if os.path.isdir(_p) and _p not in sys.path:
        sys.path.insert(0, _p)

import numpy as np

import concourse.bass as bass
import concourse.bacc as bacc
import concourse.mybir as mybir
import concourse.tile as tile
from concourse.bass_utils import run_bass_kernel_spmd
from concourse.masks import make_identity

F32 = mybir.dt.float32
BF16 = mybir.dt.bfloat16
AF = mybir.ActivationFunctionType
OP = mybir.AluOpType
AX = mybir.AxisListType

N_CORES = 8
H = 128
T = 4
F_IN = 32
EC = 16
EPS = 1e-5
GROUP_COLS = 2048

_DEG_HIST = np.array([0.0, 5000.0, 20000.0, 25000.0, 10000.0])
_BINS = np.arange(_DEG_HIST.size)
AVG_DEG_LOG = float((np.log(_BINS + 1.0) * _DEG_HIST).sum() / _DEG_HIST.sum())


def _ceil_to(x, m):
    return ((x + m - 1) // m) * m


# --------------------------------------------------------------------------
# Host-side planning (sharding + layout)
# --------------------------------------------------------------------------

class Plan:
    pass


def make_plan(src, dst, n_nodes, n_cores=N_CORES):
    assert n_nodes % n_cores == 0
    npc = n_nodes // n_cores
    p = Plan()
    p.n_nodes = n_nodes
    p.npc = npc
    p.n_cores = n_cores

    owner = dst // npc
    core_edges = []
    core_deg = []
    dmax = 0
    for c in range(n_cores):
        sel = np.nonzero(owner == c)[0]
        dloc = dst[sel] - c * npc
        deg = np.bincount(dloc, minlength=npc)
        dmax = max(dmax, int(deg.max()) if deg.size else 0)
        core_edges.append(sel)
        core_deg.append(deg)
    assert dmax <= 512, f"degree {dmax} too large"
    p.dmax = dmax

    n_d_max = np.zeros(dmax + 1, dtype=np.int64)
    for c in range(n_cores):
        cnt = np.bincount(core_deg[c], minlength=dmax + 1)
        n_d_max = np.maximum(n_d_max, cnt)

    sec_size = [int(n_d_max[0])] + [int(n_d_max[d]) for d in range(1, dmax + 1)]
    sec_off = np.concatenate([[0], np.cumsum(sec_size)])
    n_used = int(sec_off[-1])
    p.N_layout = _ceil_to(max(n_used, 512), 512)
    p.n0_max = int(n_d_max[0])
    p.n_used = n_used

    chunks = []  # (d, g, cols, slot_base, node_base)
    sbase = 0
    for d in range(1, dmax + 1):
        rem = int(n_d_max[d])
        nbase = int(sec_off[d])
        gmax = 512 // d
        while rem > 0:
            g = min(rem, gmax)
            cols = _ceil_to(g * d, 128)
            chunks.append((d, g, cols, sbase, nbase))
            sbase += cols
            nbase += g
            rem -= g
    p.chunks = chunks
    p.S = sbase if sbase > 0 else 128

    # sections: per degree-class node range (d >= 1, nonempty)
    p.sections = []
    for d in range(1, dmax + 1):
        if n_d_max[d] > 0:
            p.sections.append((d, int(sec_off[d]), int(sec_off[d + 1])))
    p.dclasses = [d for (d, a, b) in p.sections]

    # gather groups of consecutive chunks, total cols <= GROUP_COLS
    groups = []
    cur = None
    for ci, (d, g, cols, sb, nb) in enumerate(chunks):
        if cur is None or cur[1] + cols > GROUP_COLS:
            cur = [sb, cols, [ci]]
            groups.append(cur)
        else:
            cur[1] += cols
            cur[2].append(ci)
    p.groups = [tuple(x) for x in groups]

    # per-core node layout + slot->edge map (k-major within chunks)
    p.layout_nodes = []
    p.core_edges_sorted = []
    for c in range(n_cores):
        deg = core_deg[c]
        lay = np.full(p.N_layout, -1, dtype=np.int64)
        for d in range(0, dmax + 1):
            ids = np.nonzero(deg == d)[0]
            lay[sec_off[d]:sec_off[d] + ids.size] = ids
        p.layout_nodes.append(lay)

        sel = core_edges[c]
        dloc = dst[sel] - c * npc
        eorder = np.argsort(dloc, kind="stable")
        sel_sorted = sel[eorder]
        starts = np.zeros(npc + 1, dtype=np.int64)
        starts[1:] = np.cumsum(deg)

        slot_edge = np.full(p.S, -1, dtype=np.int64)
        for (d, g, cols, sb, nb) in chunks:
            nodes = lay[nb:nb + g]
            real = np.nonzero(nodes >= 0)[0]
            ed = np.full((g, d), -1, dtype=np.int64)
            if real.size:
                rn = nodes[real]
                em = starts[rn][:, None] + np.arange(d)[None, :]
                ed[real] = sel_sorted[em]
            # k-major: slot = k*g + j
            slot_edge[sb:sb + g * d] = ed.T.ravel()
        p.core_edges_sorted.append(slot_edge)

    # weight-blob index layout (structure only; values filled by make_weights)
    p.widx = dict(WXI=0, WXJ=1, W2BD=2, IDENT=3, W1X=4, W2P=5, WLINC=6)
    p.wd_idx = {}
    p.cv_idx = {0: 0}
    bi, ci = 7, 1
    for d in p.dclasses:
        p.wd_idx[d] = bi
        bi += 1 if d == 1 else 4
        p.cv_idx[d] = ci
        ci += 1
    p.n_wblocks = bi
    p.n_cvrow = 1 + ci
    return p


def make_core_inputs(p, c, atom_x, bond_x, src, W):
    npc = p.npc
    lay = p.layout_nodes[c]
    slot_edge = p.core_edges_sorted[c]
    S = p.S

    import ml_dtypes
    valid = slot_edge >= 0
    se = np.maximum(slot_edge, 0)
    xj_id = np.where(valid, src[se], 0)
    xj = atom_x[xj_id]
    xj[~valid] = 0.0
    xjT = np.ascontiguousarray(xj.T.astype(ml_dtypes.bfloat16))

    bondT = np.zeros((S, EC), dtype=ml_dtypes.bfloat16)
    bondT[valid] = bond_x[slot_edge[valid]].astype(ml_dtypes.bfloat16)
    bondT = np.ascontiguousarray(bondT.T)

    gid = np.where(lay >= 0, c * npc + lay, 0)
    xl = atom_x[gid]                                    # [NL, 128]
    x_layT = np.ascontiguousarray(xl.T.astype(ml_dtypes.bfloat16))
    x_lay = np.ascontiguousarray(
        xl.reshape(p.N_layout // 128, 128, H).transpose(1, 0, 2).reshape(128, -1))

    m = dict(xjT=xjT, bondT=bondT, x_layT=x_layT, x_lay=x_lay)
    m.update(W)
    return m


def make_weights(inp, p):
    """Host-side weight folding. Returns dict of shared DRAM inputs."""
    import ml_dtypes
    BF = ml_dtypes.bfloat16
    W_pre1, b_pre1 = np.asarray(inp["W_pre1"], np.float64), np.asarray(inp["b_pre1"], np.float64)
    W_pre2, b_pre2 = np.asarray(inp["W_pre2"], np.float64), np.asarray(inp["b_pre2"], np.float64)
    W_post1, b_post1 = np.asarray(inp["W_post1"], np.float64), np.asarray(inp["b_post1"], np.float64)
    W_post2, b_post2 = np.asarray(inp["W_post2"], np.float64), np.asarray(inp["b_post2"], np.float64)
    W_lin, b_lin = np.asarray(inp["W_lin"], np.float64), np.asarray(inp["b_lin"], np.float64)
    W_bond, b_bond = np.asarray(inp["W_bond"], np.float64), np.asarray(inp["b_bond"], np.float64)
    W_ee, b_ee = np.asarray(inp["W_ee"], np.float64), np.asarray(inp["b_ee"], np.float64)
    assert np.allclose(np.asarray(inp["ln_w"]), 1.0) and \
        np.allclose(np.asarray(inp["ln_b"]), 0.0), "ln affine not identity"

    def blockdiag(mats):
        n = len(mats)
        r, co = mats[0].shape
        out = np.zeros((n * r, n * co), dtype=np.float64)
        for t in range(n):
            out[t * r:(t + 1) * r, t * co:(t + 1) * co] = mats[t]
        return out

    W_be = W_bond @ W_ee
    b_be = b_bond @ W_ee + b_ee
    Wxi = blockdiag([W_pre1[t][0:F_IN] for t in range(T)])
    Wxj = blockdiag([W_pre1[t][F_IN:2 * F_IN] for t in range(T)])
    Wea_cat = np.concatenate([W_pre1[t][2 * F_IN:3 * F_IN] for t in range(T)], axis=1)
    W_bea = W_be @ Wea_cat
    b1p = b_pre1.reshape(H) + b_be @ Wea_cat
    W2bd = blockdiag([W_pre2[t] for t in range(T)])
    b2 = b_pre2.reshape(H)
    W1X = blockdiag([W_post1[t][0:F_IN] for t in range(T)])
    W1 = [[blockdiag([W_post1[t][F_IN + r * 5 * F_IN + a * F_IN:
                                 F_IN + r * 5 * F_IN + (a + 1) * F_IN]
                      for t in range(T)]) for a in range(5)] for r in range(3)]
    W2p = blockdiag([W_post2[t] for t in range(T)])
    b3 = b_post2.reshape(H)
    bp1 = b_post1.reshape(H)

    # LN centering fold
    P_c = np.eye(H) - np.ones((H, H)) / H
    WLINc = W_lin @ P_c
    blinc = P_c @ b_lin

    # per-degree-class combined weights + corrections
    wblocks = [Wxi, Wxj, W2bd, np.eye(H), W1X, W2p, WLINc]
    cvecs = []

    def wcomb(d):
        dc = max(d, 1.0)
        logdeg = np.log(dc + 1.0)
        amp, att = logdeg / AVG_DEG_LOG, AVG_DEG_LOG / logdeg
        return [W1[0][a] + amp * W1[1][a] + att * W1[2][a] for a in range(5)]

    # class 0 = degree 0: std = sqrt(eps), everything else zero
    Wc0 = wcomb(0)
    cvecs.append(np.sqrt(EPS) * Wc0[4].sum(axis=0))
    for d in p.dclasses:
        Wc = wcomb(d)
        if d == 1:
            wblocks.append(Wc[0] + Wc[1] + Wc[2] + Wc[3])
            cvecs.append(b2 @ (1 * Wc[0] + Wc[1] + Wc[2] + Wc[3])
                         + np.sqrt(EPS) * Wc[4].sum(axis=0))
        else:
            wblocks.extend([Wc[0] + Wc[1] / d, Wc[2], Wc[3], Wc[4]])
            cvecs.append(b2 @ (d * Wc[0] + Wc[1] + Wc[2] + Wc[3]))

    assert len(wblocks) == p.n_wblocks
    wmm = np.concatenate([np.asarray(w) for w in wblocks], axis=1).astype(BF)
    cvrow = np.concatenate([np.ones(H)] + cvecs).astype(BF)[None, :]
    assert cvrow.shape[1] == p.n_cvrow * 128, (cvrow.shape, p.n_cvrow)
    baux = np.stack([b1p, bp1, b3, blinc], axis=1).astype(np.float32)  # [128,4]
    return dict(
        wmm=np.ascontiguousarray(wmm),
        wbea16=np.ascontiguousarray(W_bea.astype(BF)),
        cvrow=np.ascontiguousarray(cvrow),
        baux=np.ascontiguousarray(baux),
    )


# --------------------------------------------------------------------------
# Bass kernel builder
# --------------------------------------------------------------------------

def build_nc(p, n_nodes, debug=False):
    nc = bacc.Bacc("TRN2", target_bir_lowering=False, debug=debug)
    S, NL = p.S, p.N_layout
    NB = NL // 512

    xjT_d = nc.dram_tensor("xjT", [128, S], BF16, kind="ExternalInput")
    bondT_d = nc.dram_tensor("bondT", [EC, S], BF16, kind="ExternalInput")
    xlt_d = nc.dram_tensor("x_layT", [128, NL], BF16, kind="ExternalInput")
    xln_d = nc.dram_tensor("x_lay", [128, NL], F32, kind="ExternalInput")
    wmm_d = nc.dram_tensor("wmm", [128, p.n_wblocks * 128], BF16, kind="ExternalInput")
    wbea_d = nc.dram_tensor("wbea16", [EC, 128], BF16, kind="ExternalInput")
    cvrow_d = nc.dram_tensor("cvrow", [1, p.n_cvrow * 128], BF16, kind="ExternalInput")
    baux_d = nc.dram_tensor("baux", [128, 4], F32, kind="ExternalInput")
    out_d = nc.dram_tensor("out", [NL, H], F32, kind="ExternalOutput")

    # blocks -> pieces; piece = (d, a, b) with [a,b) within block
    sec_all = [(0, 0, p.n0_max)] + p.sections
    block_pieces = [[] for _ in range(NB)]
    for (d, a, b) in sec_all:
        if b <= a:
            continue
        t0, t1 = a // 512, (b - 1) // 512
        for t in range(t0, t1 + 1):
            pa, pb_ = max(a, t * 512), min(b, (t + 1) * 512)
            if pb_ > pa:
                block_pieces[t].append((d, pa, pb_))

    # block -> last chunk index contributing to it
    last_chunk = [-1] * NB
    for ci, (d, g, cols, sb, nb) in enumerate(p.chunks):
        for t in range(nb // 512, min((nb + g - 1) // 512, NB - 1) + 1):
            last_chunk[t] = max(last_chunk[t], ci)
    # ensure monotone (a block can't be ready before an earlier block's chunks)
    for t in range(1, NB):
        last_chunk[t] = max(last_chunk[t], last_chunk[t - 1])

    with tile.TileContext(nc) as tc:
        from contextlib import ExitStack
        with ExitStack() as ctx:
            cpool = ctx.enter_context(tc.tile_pool(name="consts", bufs=1))
            wmm = cpool.tile([128, p.n_wblocks * 128], BF16)
            nc.sync.dma_start(wmm[:], wmm_d[:])
            wbea16 = cpool.tile([EC, 128], BF16)
            nc.sync.dma_start(wbea16[:], wbea_d[:])
            cvrow = cpool.tile([1, p.n_cvrow * 128], BF16)
            nc.sync.dma_start(cvrow[:], cvrow_d[:])
            baux = cpool.tile([128, 4], F32)
            nc.sync.dma_start(baux[:], baux_d[:])
            ident = cpool.tile([128, 128], F32)
            make_identity(nc, ident[:])
            epsc = cpool.tile([128, 1], F32)
            nc.vector.memset(epsc[:], EPS)
            onescol = cpool.tile([128, 1], BF16)
            nc.vector.memset(onescol[:], 1.0)
            ones512 = cpool.tile([1, 512], BF16)
            nc.vector.memset(ones512[:], 1.0)

            def WB(i):
                return wmm[:, i * 128:(i + 1) * 128]

            def CV(di):
                i = p.cv_idx[di] + 1  # +1: row block 0 is the ones row
                return cvrow[:, i * 128:(i + 1) * 128]

            ONESROW = cvrow[:, 0:128]
            B1P, BP1, B3, BLINC = (baux[:, i:i + 1] for i in range(4))

            # persistent aggregate arrays
            aggp = ctx.enter_context(tc.tile_pool(name="agg", bufs=1))
            s16_agg = aggp.tile([128, NL], BF16)
            mn16_agg = aggp.tile([128, NL], BF16)
            mx16_agg = aggp.tile([128, NL], BF16)
            std16_agg = aggp.tile([128, NL], BF16)

            ep = ctx.enter_context(tc.tile_pool(name="edge_sb", bufs=4))
            sp2 = ctx.enter_context(tc.tile_pool(name="stdchain_sb", bufs=2))
            gp = ctx.enter_context(tc.tile_pool(name="edge_gath", bufs=3))
            pb = ctx.enter_context(tc.tile_pool(name="post_sb", bufs=2))
            epp = ctx.enter_context(tc.tile_pool(name="edge_ps", bufs=4, space="PSUM"))
            sqp = ctx.enter_context(tc.tile_pool(name="sq_ps", bufs=2, space="PSUM"))
            opp = ctx.enter_context(tc.tile_pool(name="op_ps", bufs=1, space="PSUM"))
            mpp = ctx.enter_context(tc.tile_pool(name="misc_ps", bufs=1, space="PSUM"))

            gtiles = {}

            def emit_gather(gi):
                gsb, gcols, _ = p.groups[gi]
                xj_fmg = gp.tile([128, gcols], BF16, tag="xj_fmg")
                gtiles[gi] = xj_fmg
                nc.sync.dma_start(xj_fmg[:], xjT_d[:, gsb:gsb + gcols])

            def emit_head(gi, ci):
                """Independent front of a chunk: p1 matmuls + h1 evac.

                Emitted one chunk ahead of the dependent tail so the in-order
                PE queue always has ready work in front of tail matmuls that
                wait on the scalar h1 evacuation."""
                gsb, gcols, _ = p.groups[gi]
                xj_fmg = gtiles[gi]
                d, g, cols, sb, nb = p.chunks[ci]
                loc = sb - gsb
                gd = g * d
                nsl = slice(nb, nb + g)

                bond_t = ep.tile([EC, 512], BF16, tag="bond")
                nc.sync.dma_start(bond_t[:, 0:gd], bondT_d[:, sb:sb + gd])
                xi_t = ep.tile([128, 512], BF16, tag="xi_t")
                nc.sync.dma_start(xi_t[:, 0:g], xlt_d[:, nsl])

                p1 = epp.tile([128, 512], F32, tag="mm")
                nc.tensor.matmul(out=p1[:, 0:gd], lhsT=wbea16[:],
                                 rhs=bond_t[:, 0:gd], start=True, stop=False)
                nc.tensor.matmul(out=p1[:, 0:gd], lhsT=WB(p.widx["WXJ"]),
                                 rhs=xj_fmg[:, loc:loc + gd], start=False, stop=False)
                nc.tensor.matmul(
                    out=p1[:, 0:gd].rearrange("p (k j) -> p k j", k=d),
                    lhsT=WB(p.widx["WXI"]),
                    rhs=xi_t[:, None, 0:g].to_broadcast((128, d, g)),
                    start=False, stop=True)
                h1 = ep.tile([128, 512], BF16, tag="h1")
                nc.scalar.activation(h1[:, 0:gd], p1[:, 0:gd], AF.Relu, bias=B1P)
                return h1

            def emit_tail(ci, h1):
                d, g, cols, sb, nb = p.chunks[ci]
                gd = g * d
                nsl = slice(nb, nb + g)
                p2 = epp.tile([128, 512], F32, tag="mm")
                nc.tensor.matmul(out=p2[:, 0:gd], lhsT=WB(p.widx["W2BD"]),
                                 rhs=h1[:, 0:gd], start=True, stop=True)

                if d == 1:
                    nc.scalar.activation(s16_agg[:, nsl], p2[:, 0:g], AF.Copy)
                    return

                msq = ep.tile([128, 512], BF16, tag="msq")
                nc.scalar.activation(msq[:, 0:gd], p2[:, 0:gd], AF.Square)

                sq = sqp.tile([128, 512], F32, tag="sq")
                nc.tensor.matmul(
                    out=sq[:, None, 0:g].to_broadcast((128, d, g)),
                    lhsT=WB(p.widx["W2BD"]),
                    rhs=h1[:, 0:gd].rearrange("p (k j) -> p k j", k=d),
                    start=True, stop=True, skip_group_check=True)
                nc.tensor.matmul(
                    out=sq[:, None, 256:256 + g].to_broadcast((128, d, g)),
                    lhsT=WB(p.widx["IDENT"]),
                    rhs=msq[:, 0:gd].rearrange("p (k j) -> p k j", k=d),
                    start=True, stop=True, skip_group_check=True)

                nc.scalar.activation(s16_agg[:, nsl], sq[:, 0:g], AF.Copy)
                msqt_t = sp2.tile([128, 256], F32, tag="msqt")
                nc.scalar.activation(msqt_t[:, 0:g], sq[:, 0:g], AF.Square,
                                     scale=1.0 / d)
                e2t = sp2.tile([128, 256], F32, tag="e2t")
                nc.scalar.activation(e2t[:, 0:g], sq[:, 256:256 + g], AF.Copy,
                                     scale=1.0 / d)
                vt = sp2.tile([128, 256], F32, tag="vt")
                nc.gpsimd.tensor_tensor(vt[:, 0:g], e2t[:, 0:g], msqt_t[:, 0:g],
                                        OP.subtract)
                vt2 = sp2.tile([128, 256], F32, tag="vt2")
                # (var max 0) + eps in one DVE op; then a plain sqrt
                nc.vector.tensor_scalar(vt2[:, 0:g], vt[:, 0:g], 0.0, EPS,
                                        OP.max, OP.add)
                nc.scalar.activation(std16_agg[:, nsl], vt2[:, 0:g], AF.Sqrt)

                p2v = p2[:, 0:gd].rearrange("p (k j) -> p j k", k=d)
                nc.vector.tensor_reduce(out=mn16_agg[:, nsl], in_=p2v,
                                        axis=AX.X, op=OP.min)
                nc.vector.tensor_reduce(out=mx16_agg[:, nsl], in_=p2v,
                                        axis=AX.X, op=OP.max)

            def emit_block(t):
                nb = t * 512
                nsl = slice(nb, nb + 512)
                pieces = block_pieces[t]

                x_fm = pb.tile([128, 512], BF16, tag="x_fm")
                nc.sync.dma_start(x_fm[:], xlt_d[:, nsl])
                x_nm = pb.tile([128, 512], F32, tag="x_nm")
                nc.sync.dma_start(x_nm[:], xln_d[:, nsl])

                op = opp.tile([128, 512], F32, tag="op")
                mms = [dict(out=op[:], lhsT=WB(p.widx["W1X"]), rhs=x_fm[:])]
                for (d, a, b) in pieces:
                    rel = slice(a - nb, b - nb)
                    n = b - a
                    if d == 0:
                        mms.append(dict(out=op[:, rel], lhsT=CV(0),
                                        rhs=ones512[:, 0:n]))
                    elif d == 1:
                        wi = p.wd_idx[1]
                        mms.append(dict(out=op[:, rel], lhsT=WB(wi),
                                        rhs=s16_agg[:, a:b]))
                        mms.append(dict(out=op[:, rel], lhsT=CV(1),
                                        rhs=ones512[:, 0:n]))
                    else:
                        wi = p.wd_idx[d]
                        mms.append(dict(out=op[:, rel], lhsT=WB(wi),
                                        rhs=s16_agg[:, a:b]))
                        mms.append(dict(out=op[:, rel], lhsT=WB(wi + 1),
                                        rhs=mn16_agg[:, a:b]))
                        mms.append(dict(out=op[:, rel], lhsT=WB(wi + 2),
                                        rhs=mx16_agg[:, a:b]))
                        mms.append(dict(out=op[:, rel], lhsT=WB(wi + 3),
                                        rhs=std16_agg[:, a:b]))
                        mms.append(dict(out=op[:, rel], lhsT=CV(d),
                                        rhs=ones512[:, 0:n]))
                for i, kw in enumerate(mms):
                    nc.tensor.matmul(start=(i == 0), stop=(i == len(mms) - 1),
                                     skip_group_check=True, **kw)

                h1p = pb.tile([128, 512], BF16, tag="h1p")
                nc.vector.tensor_scalar(h1p[:], op[:], BP1, 0.0, OP.add, OP.max)
                pp2 = mpp.tile([128, 512], F32, tag="mp")
                nc.tensor.matmul(out=pp2[:], lhsT=WB(p.widx["W2P"]), rhs=h1p[:],
                                 start=True, stop=True)
                z2 = pb.tile([128, 512], BF16, tag="z2")
                nc.vector.tensor_scalar(z2[:], pp2[:], B3, None, OP.add)
                plin = mpp.tile([128, 512], F32, tag="mp")
                nc.tensor.matmul(out=plin[:], lhsT=WB(p.widx["WLINC"]), rhs=z2[:],
                                 start=True, stop=True)
                zf = pb.tile([128, 512], F32, tag="zf")
                nc.scalar.activation(zf[:], plin[:], AF.Identity, bias=BLINC)
                sq16 = pb.tile([128, 512], BF16, tag="sq16")
                nc.gpsimd.tensor_tensor(sq16[:], zf[:], zf[:], OP.mult)
                vs = mpp.tile([128, 512], F32, tag="mp")
                nc.tensor.matmul(out=vs[0:1, :], lhsT=onescol[:], rhs=sq16[:],
                                 start=True, stop=True)
                sd = pb.tile([1, 512], F32, tag="sd")
                nc.scalar.activation(sd[:], vs[0:1, :], AF.Sqrt,
                                     scale=1.0 / 128.0, bias=epsc[0:1, :])
                ri32 = pb.tile([1, 512], F32, tag="ri32")
                nc.vector.reciprocal_approx_fast(ri32[:], sd[:])
                ri = pb.tile([1, 512], BF16, tag="ri")
                with nc.allow_low_precision(reason="rstd broadcast via bf16 ones matmul"):
                    nc.vector.tensor_copy(ri[:], ri32[:])
                rb = mpp.tile([128, 512], F32, tag="mp")
                nc.tensor.matmul(out=rb[:], lhsT=ONESROW, rhs=ri[:],
                                 start=True, stop=True)
                y = pb.tile([128, 512], F32, tag="y")
                nc.vector.tensor_tensor(y[:], zf[:], rb[:], OP.mult)
                ry = pb.tile([128, 512], F32, tag="ry")
                nc.vector.tensor_scalar(ry[:], y[:], 0.0, None, OP.max)
                zps = mpp.tile([128, 512], F32, tag="mp")
                for bq in range(4):
                    sl = slice(128 * bq, 128 * (bq + 1))
                    nc.tensor.transpose(out=zps[:, sl], in_=ry[:, sl],
                                        identity=ident[:])
                out_nm = pb.tile([128, 512], F32, tag="out_nm")
                nc.vector.tensor_tensor(out_nm[:], zps[:], x_nm[:], OP.add)
                nc.sync.dma_start(
                    out_d[nsl, :].rearrange("(j p) f -> p j f", p=128),
                    out_nm[:].rearrange("p (j f) -> p j f", f=128))

            # interleaved, software-pipelined emission: head(i+1) before tail(i)
            next_block = 0

            def emit_tail_and_blocks(item):
                nonlocal next_block
                ci, h1 = item
                emit_tail(ci, h1)
                while next_block < NB and last_chunk[next_block] <= ci:
                    emit_block(next_block)
                    next_block += 1

            DEPTH = 3
            pending = []  # [(ci, h1), ...] awaiting tails
            for gi in range(len(p.groups)):
                emit_gather(gi)
                for ci in p.groups[gi][2]:
                    h1 = emit_head(gi, ci)
                    pending.append((ci, h1))
                    if len(pending) > DEPTH:
                        emit_tail_and_blocks(pending.pop(0))
            for item in pending:
                emit_tail_and_blocks(item)
            while next_block < NB:
                emit_block(next_block)
                next_block += 1
    nc.compile()
    return nc


# --------------------------------------------------------------------------
# Entry point
# --------------------------------------------------------------------------

_CACHE = {}


def _get_compiled(src, dst, n_nodes):
    key = hash((src.tobytes(), dst.tobytes(), n_nodes))
    if key not in _CACHE:
        p = make_plan(src.astype(np.int64), dst.astype(np.int64), n_nodes)
        nc = build_nc(p, n_nodes)
        _CACHE[key] = (p, nc)
    return _CACHE[key]


def kernel(**inputs):
    atom_x = np.asarray(inputs["atom_x"], np.float32)
    bond_x = np.asarray(inputs["bond_x"], np.float32)
    ei = np.asarray(inputs["atom_edge_index"])
    src = ei[0].astype(np.int64)
    n_nodes = atom_x.shape[0]

    p, nc = _get_compiled(ei[0], ei[1], n_nodes)
    W = make_weights(inputs, p)
    in_maps = [make_core_inputs(p, c, atom_x, bond_x, src, W)
               for c in range(p.n_cores)]
    res = run_bass_kernel_spmd(nc, in_maps, core_ids=list(range(p.n_cores)))

    out = np.zeros((n_nodes, H), dtype=np.float32)
    for c in range(p.n_cores):
        o = res.results[c]["out"]
        lay = p.layout_nodes[c]
        real = np.nonzero(lay >= 0)[0]
        out[c * p.npc + lay[real]] = o[real]
    return out
